# revision 12
# baseline (speedup 1.0000x reference)
"""Trainium2 Bass kernel for ADRiverDynamics (gnn_message_passing).

8 independent point clouds (B*L=8), one per NeuronCore (pure data parallel),
plus one tiny AllReduce for global BatchNorm statistics.

Per-core pipeline (cloud of N=3072 points, C=64 channels, K=16 neighbors):
  S0  load f/xyz, weights
  S1  PE transposes (fxT = [f|xyz] channel-major), head convs, gate conv;
      bf16 3-way-split tensors A36/B36 for the distance matmul (one 36-row
      bf16 matmul per 512-chunk instead of a 4-pass fp32 matmul; the split
      keeps fp32-grade accuracy: x = h+m+l with h,m,l bf16 and a 4th
      augmented coordinate carrying (1, -sq_j))
  S2  pass A: negd' = 2 x_i.x_j - sq_j accumulated in PSUM; per-row top-16
      of 3072 via 8-way segmented Max/MaxIndex reading PSUM directly (the
      per-row -sq_i shift is order-invariant and fixed up later); merged via
      match_replace + gpsimd rank trick
  S3  idx staged through DRAM into the wrapped per-core gather layout
  S4  C1: per slice gather f/xyz columns (fp32), PE transposes back to
      point-major, Act copies into an fp16 k-minor fnei buffer
  S5  C2 (batched): d2 from saved top-k values (no gather), sqrt/exp in two
      table loads, softmax weights, fp16 2x-mode weighted tree aggregation
  S6  reaction conv + global-batch BN (AllReduce) + relu + conv, combine
"""
import functools
import numpy as np

B, L, N, C, K = 2, 4, 3072, 64, 16
NB = N // 128          # 24 point blocks
TAU = 0.15
BN_EPS = 1e-5
NCORES = 8
BT = 4                 # blocks per gather slice
NSL = NB // BT         # gather slices

WEIGHT_NAMES = ["Wf", "bf", "Wd", "bd", "Wu", "bu", "Wg1", "bg1", "Wg2", "bg2",
                "Wgate", "bgate", "Wr1", "br1", "gamma", "beta", "Wr2", "br2",
                "log_dt"]


def _build(debug=False, nocol=False):
    import contextlib
    from concourse import bacc
    import concourse.bass as bass
    import concourse.tile as tile
    import concourse.mybir as mybir
    from concourse import masks

    f32 = mybir.dt.float32
    bf16 = mybir.dt.bfloat16
    f16 = mybir.dt.float16
    u16 = mybir.dt.uint16
    i16 = mybir.dt.int16
    Alu = mybir.AluOpType
    Act = mybir.ActivationFunctionType
    AX = mybir.AxisListType
    AP = bass.AP

    nc = bacc.Bacc("TRN2", target_bir_lowering=False, debug=False,
                   num_devices=NCORES)

    f_ext = nc.dram_tensor("f", [N, C], f32, kind="ExternalInput")
    xyz_ext = nc.dram_tensor("xyz", [N, 3], f32, kind="ExternalInput")
    wshapes = {"Wf": [3, C], "bf": [3], "Wd": [1, C], "bd": [1], "Wu": [1, C],
               "bu": [1], "Wg1": [C, 3], "bg1": [C], "Wg2": [C, C], "bg2": [C],
               "Wgate": [C, C], "bgate": [C], "Wr1": [C, C + 5], "br1": [C],
               "gamma": [C], "beta": [C], "Wr2": [C, C], "br2": [C],
               "log_dt": [1]}
    w_ext = {k: nc.dram_tensor(k, shp, f32, kind="ExternalInput")
             for k, shp in wshapes.items()}
    out_ext = nc.dram_tensor("out", [N, C], f32, kind="ExternalOutput")
    dbg_ext = {}
    if debug:
        for k, shp in {"d_idx": [128, NB * K], "d_agg": [128, NB * C],
                       "d_uw": [128, NB * K], "d_v16": [128, NB * K],
                       "d_numv": [128, NB * K], "d_heads": [5, N]}.items():
            dbg_ext[k] = nc.dram_tensor(k, shp, f32, kind="ExternalOutput")

    with tile.TileContext(nc) as tc:
        class _Stacks(contextlib.ExitStack):
            def __init__(self):
                super().__init__()
                self._pa = contextlib.ExitStack()
                self._pc = contextlib.ExitStack()
            def enter_pa(self, cm):
                return self._pa.enter_context(cm)
            def enter_pc(self, cm):
                return self._pc.enter_context(cm)
            def close_pa(self):
                self._pa.close()
            def close_pc(self):
                self._pc.close()
            def __exit__(self, *a):
                self._pc.close()
                self._pa.close()
                return super().__exit__(*a)
        ctx = _Stacks()
        with ctx:
            cpool = ctx.enter_context(tc.tile_pool(name="consts", bufs=1))
            big = ctx.enter_context(tc.tile_pool(name="big", bufs=1))
            dram = ctx.enter_context(tc.tile_pool(name="dram", bufs=1, space="DRAM"))
            psum = ctx.enter_context(tc.tile_pool(name="psum", bufs=2, space="PSUM"))
            negps = ctx.enter_context(tc.tile_pool(name="negps", bufs=2, space="PSUM"))
            small = ctx.enter_context(tc.tile_pool(name="small", bufs=1))

            def ps(p, fr):
                return psum.tile([p, fr], f32, tag="ps", name="pst")

            # ---------------- constants / weights -----------------
            ident = cpool.tile([128, 128], f32)
            masks.make_identity(nc, ident[:])

            WhT = cpool.tile([C, 5], f32)
            nc.sync.dma_start(WhT[:, 0:3], AP(w_ext["Wf"], 0, [[1, C], [C, 3]]))
            nc.sync.dma_start(WhT[:, 3:4], AP(w_ext["Wd"], 0, [[1, C], [C, 1]]))
            nc.sync.dma_start(WhT[:, 4:5], AP(w_ext["Wu"], 0, [[1, C], [C, 1]]))
            bhead = cpool.tile([5, 1], f32)
            nc.sync.dma_start(bhead[0:3, :], AP(w_ext["bf"], 0, [[1, 3], [1, 1]]))
            nc.sync.dma_start(bhead[3:4, :], AP(w_ext["bd"], 0, [[1, 1], [1, 1]]))
            nc.sync.dma_start(bhead[4:5, :], AP(w_ext["bu"], 0, [[1, 1], [1, 1]]))

            WgateT = cpool.tile([C, C], f32)
            nc.sync.dma_start(WgateT[:], AP(w_ext["Wgate"], 0, [[1, C], [C, C]]))
            Wg1T = cpool.tile([3, C], f32)
            nc.sync.dma_start(Wg1T[:], AP(w_ext["Wg1"], 0, [[1, 3], [3, C]]))
            Wg2T = cpool.tile([C, C], f32)
            nc.sync.dma_start(Wg2T[:], AP(w_ext["Wg2"], 0, [[1, C], [C, C]]))
            Wr1fT = cpool.tile([C, C], f32)
            nc.sync.dma_start(Wr1fT[:], AP(w_ext["Wr1"], 0, [[1, C], [C + 5, C]]))
            Wr1hT = cpool.tile([5, C], f32)
            nc.sync.dma_start(Wr1hT[:], AP(w_ext["Wr1"], C, [[1, 5], [C + 5, C]]))
            Wr2T = cpool.tile([C, C], f32)
            nc.sync.dma_start(Wr2T[:], AP(w_ext["Wr2"], 0, [[1, C], [C, C]]))

            def vec_col(name):
                t = cpool.tile([C, 1], f32, tag=name, name=name + "_v")
                nc.sync.dma_start(t[:], AP(w_ext[name], 0, [[1, C], [1, 1]]))
                return t
            bgate_v = vec_col("bgate")
            bg1_v = vec_col("bg1")
            bg2_v = vec_col("bg2")
            br2_v = vec_col("br2")
            gamma_v = vec_col("gamma")
            beta_v = vec_col("beta")

            zero128 = cpool.tile([128, 1], f32)
            nc.vector.memset(zero128[:], 0.0)
            segb64u = cpool.tile([128, 64], u16)
            nc.gpsimd.iota(segb64u[:], pattern=[[384, 8], [0, 8]],
                           channel_multiplier=0)
            rank16 = cpool.tile([128, 16], i16)
            nc.gpsimd.iota(rank16[:], pattern=[[1, 16]], base=1,
                           channel_multiplier=0)

            dtv = cpool.tile([128, 1], f32)

            # ---------------- S0 loads -----------------
            f_sb = big.tile([128, NB, C], f32)
            nc.sync.dma_start(f_sb[:], AP(f_ext, 0, [[C, 128], [128 * C, NB], [1, C]]))
            xyz_sb = big.tile([128, NB, 3], f32)
            nc.sync.dma_start(xyz_sb[:], AP(xyz_ext, 0, [[3, 128], [128 * 3, NB], [1, 3]]))

            # ---------------- S1 transposes + convs -----------------
            # fxT: rows 0:64 f, 64:67 xyz (fp32, the gather source + conv input)
            fxT = big.tile([128, N], f32)
            fT = fxT[0:C, :]
            for j in range(6):
                pt = ps(C, 512)
                for q in range(4):
                    b = 4 * j + q
                    nc.tensor.matmul(pt[:, 128 * q:128 * (q + 1)],
                                     f_sb[:, b:b + 1, :], ident[:, :],
                                     is_transpose=True)
                nc.scalar.copy(fxT[0:C, 512 * j:512 * (j + 1)], pt[:])
            for j in range(6):
                pt = ps(3, 512)
                for q in range(4):
                    b = 4 * j + q
                    nc.tensor.matmul(pt[:, 128 * q:128 * (q + 1)],
                                     xyz_sb[:, b:b + 1, :], ident[:, :],
                                     is_transpose=True)
                nc.scalar.copy(fxT[C:C + 3, 512 * j:512 * (j + 1)], pt[:])

            # --- split tensors for the distance matmul ---
            s1stk = contextlib.ExitStack()
            spl = s1stk.enter_context(tc.tile_pool(name="split", bufs=1))
            pmA = spl.tile([128, NB, 128], bf16)
            pmB = spl.tile([128, NB, 128], bf16)
            nc.gpsimd.memset(pmA[:], 0.0)
            nc.gpsimd.memset(pmB[:], 0.0)

            rt2 = float(np.sqrt(2.0))
            yv = spl.tile([128, NB, 3], f32)
            nc.vector.tensor_scalar(yv[:], xyz_sb[:], rt2, None, Alu.mult)
            x2 = spl.tile([128, NB, 3], f32)
            nc.vector.tensor_tensor(x2[:], xyz_sb[:], xyz_sb[:], Alu.mult)
            sq_p = small.tile([128, NB, 1], f32)
            nc.vector.tensor_reduce(sq_p[:], x2[:], axis=AX.X, op=Alu.add)
            nsq = spl.tile([128, NB, 1], f32)
            nc.vector.tensor_scalar(nsq[:], sq_p[:], -1.0, None, Alu.mult)

            r3 = spl.tile([128, NB, 3], f32)
            r3b = spl.tile([128, NB, 3], f32)
            rs = spl.tile([128, NB, 1], f32)
            rs2 = spl.tile([128, NB, 1], f32)
            for pm, aug in ((pmA, None), (pmB, nsq)):
                nc.vector.tensor_copy(pm[:, :, 0:3], yv[:])
                nc.vector.tensor_tensor(r3[:], yv[:], pm[:, :, 0:3], Alu.subtract)
                nc.vector.tensor_copy(pm[:, :, 4:7], r3[:])
                nc.vector.tensor_tensor(r3b[:], r3[:], pm[:, :, 4:7], Alu.subtract)
                nc.vector.tensor_copy(pm[:, :, 8:11], r3b[:])
                if aug is None:
                    nc.vector.memset(pm[:, :, 3:4], 1.0)
                else:
                    nc.vector.tensor_copy(pm[:, :, 3:4], aug[:])
                    nc.vector.tensor_tensor(rs[:], aug[:], pm[:, :, 3:4], Alu.subtract)
                    nc.vector.tensor_copy(pm[:, :, 7:8], rs[:])
                    nc.vector.tensor_tensor(rs2[:], rs[:], pm[:, :, 7:8], Alu.subtract)
                    nc.vector.tensor_copy(pm[:, :, 11:12], rs2[:])

            TA = spl.tile([128, N], bf16)
            TB = spl.tile([128, N], bf16)
            for b in range(NB):
                nc.sync.dma_start_transpose(TA[:, 128 * b:128 * (b + 1)], pmA[:, b, :])
                nc.sync.dma_start_transpose(TB[:, 128 * b:128 * (b + 1)], pmB[:, b, :])
            # A36 rows: [Ah(4) x3, Am(4) x3, Al(4) x3]; B36: [Bh, Bm, Bl](12) x3
            A36 = big.tile([36, N], bf16)
            B36 = big.tile([36, N], bf16)
            for t in range(3):
                for p in range(3):
                    nc.sync.dma_start(A36[12 * p + 4 * t:12 * p + 4 * t + 4, :],
                                      TA[4 * p:4 * p + 4, :])
                nc.sync.dma_start(B36[12 * t:12 * (t + 1), :], TB[0:12, :])

            # --- head + gate convs ---
            headsT = big.tile([5, N], f32)
            gateT = s1stk.enter_context(tc.tile_pool(name="gate", bufs=1)).tile([C, N], f32)
            for j in range(6):
                sl = slice(512 * j, 512 * (j + 1))
                ph = ps(5, 512)
                nc.tensor.matmul(ph[:], WhT[:], fT[:, sl], start=True, stop=True)
                nc.scalar.activation(headsT[:, sl], ph[:], Act.Identity,
                                     bias=bhead[:], scale=1.0)
                pg = ps(C, 512)
                nc.tensor.matmul(pg[:], WgateT[:], fT[:, sl], start=True, stop=True)
                nc.scalar.activation(gateT[:, sl], pg[:], Act.Sigmoid,
                                     bias=bgate_v[:], scale=1.0)

            hp = small.tile([128, NB, 5], f32)
            pt5 = ps(128, NB * 5)
            for b in range(NB):
                nc.tensor.matmul(pt5[:, 5 * b:5 * (b + 1)],
                                 headsT[:, 128 * b:128 * (b + 1)], ident[0:5, 0:5],
                                 is_transpose=True)
            nc.vector.tensor_copy(hp[:], pt5[:])

            flow_p = hp[:, :, 0:3]
            # de = softplus(dpre) * (1 + sigmoid(upre))   [sigmoid then ln+exp]
            de = small.tile([128, NB, 1], f32)
            sgu = small.tile([128, NB, 1], f32)
            nc.scalar.activation(sgu[:], hp[:, :, 4:5], Act.Sigmoid,
                                 bias=zero128[:], scale=1.0)
            nc.vector.tensor_scalar(sgu[:], sgu[:], 1.0, None, Alu.add)
            tmp_b = small.tile([128, NB, 1], f32)
            nc.scalar.activation(tmp_b[:], hp[:, :, 3:4], Act.Exp,
                                 bias=zero128[:], scale=1.0)
            nc.vector.tensor_scalar(tmp_b[:], tmp_b[:], 1.0, None, Alu.add)
            nc.scalar.activation(tmp_b[:], tmp_b[:], Act.Ln,
                                 bias=zero128[:], scale=1.0)
            nc.vector.tensor_tensor(de[:], tmp_b[:], sgu[:], Alu.mult)
            de16 = small.tile([128, NB, 1], f32)
            nc.vector.tensor_scalar(de16[:], de[:], 1.0 / K, None, Alu.mult)

            nc.sync.dma_start(dtv[:], AP(w_ext["log_dt"], 0, [[0, 128], [1, 1]]))
            nc.scalar.activation(dtv[:], dtv[:], Act.Exp, bias=zero128[:], scale=1.0)
            nc.vector.tensor_scalar(dtv[:], dtv[:], 1e-4, 10.0, Alu.max, Alu.min)

            # flow normalization (sqrt table)
            fl2 = small.tile([128, NB, 3], f32)
            nc.vector.tensor_tensor(fl2[:], flow_p, flow_p, Alu.mult)
            vn = small.tile([128, NB, 1], f32)
            nc.vector.tensor_reduce(vn[:], fl2[:], axis=AX.X, op=Alu.add)
            nc.scalar.activation(vn[:], vn[:], Act.Sqrt, bias=zero128[:], scale=1.0)
            nc.vector.tensor_scalar(vn[:], vn[:], 1e-6, None, Alu.max)
            rv = small.tile([128, NB, 1], f32)
            nc.vector.reciprocal(rv[:], vn[:])
            vhat16 = small.tile([128, NB, 3], f16)
            nc.vector.tensor_tensor(vhat16[:], flow_p,
                                    rv[:].broadcast_to((128, NB, 3)), Alu.mult)
            xyz16 = small.tile([128, NB, 3], f16)
            nc.vector.tensor_copy(xyz16[:], xyz_sb[:])

            # global advection gate: fgm = mean(flow) over all points
            ones128 = cpool.tile([128, 1], f32)
            nc.vector.memset(ones128[:], 1.0)
            pfg = ps(1, NB * 5)
            nc.tensor.matmul(pfg[:], ones128[:], hp[:].rearrange("p a b -> p (a b)"),
                             start=True, stop=True)
            fgrow = small.tile([1, NB, 5], f32)
            nc.vector.tensor_copy(fgrow[:], pfg[:])
            fgm_r = small.tile([1, 5], f32)
            nc.vector.tensor_reduce(
                fgm_r[:], fgrow[:].transpose([0, 2, 1]),
                axis=AX.X, op=Alu.add)
            nc.vector.tensor_scalar(fgm_r[:], fgm_r[:], 1.0 / N, None, Alu.mult)
            pft = ps(5, 1)
            nc.tensor.matmul(pft[:], fgm_r[0:1, :], ones128[0:1, 0:1],
                             is_transpose=True)
            fgm = small.tile([5, 1], f32)
            nc.vector.tensor_copy(fgm[:], pft[:])
            pg1 = ps(C, 1)
            nc.tensor.matmul(pg1[:], Wg1T[:], fgm[0:3, :], start=True, stop=True)
            hg = small.tile([C, 1], f32)
            nc.scalar.activation(hg[:], pg1[:], Act.Relu, bias=bg1_v[:], scale=1.0)
            pg2 = ps(C, 1)
            nc.tensor.matmul(pg2[:], Wg2T[:], hg[:], start=True, stop=True)
            fgf = small.tile([C, 1], f32)
            nc.vector.tensor_scalar(fgf[:], pg2[:], bg2_v[:], None, Alu.add)
            # TR = gate * fgf, transposed to point layout
            nc.vector.tensor_scalar(gateT[:], gateT[:], fgf[:], None, Alu.mult)
            TRp = big.tile([128, NB, C], f32)
            for j in range(3):
                pt = ps(128, 512)
                for q in range(8):
                    b = 8 * j + q
                    nc.tensor.matmul(pt[:, C * q:C * (q + 1)],
                                     gateT[:, 128 * b:128 * (b + 1)],
                                     ident[0:C, 0:C], is_transpose=True)
                nc.scalar.copy(TRp[:, 8 * j:8 * (j + 1), :], pt[:])

            # ---------------- S2 pass A -----------------
            s1stk.close()
            idx_all = big.tile([128, NB * K], u16)
            v16_all = big.tile([128, NB, K], f32)
            NBG = NB // BT
            idx_dram = dram.tile([NBG * K * 512], i16)
            fnei = big.tile([128, NB, 67, K], f16)
            numv_all = big.tile([128, NB, K], f32)
            gpool = ctx.enter_pa(tc.tile_pool(name="gth", bufs=2))
            idx_wrap = big.tile([128, NBG * K, 32], i16)

            for b in range(NB):
                cand = small.tile([128, 64], f32, tag="cand", bufs=2)
                segloc = small.tile([128, 64], u16, tag="segloc", bufs=2)
                for h in range(2):
                    negd = negps.tile([128, 1536], f32, tag="negd")
                    for j in range(3):
                        cj = 3 * h + j
                        nc.tensor.matmul(negd[:, 512 * j:512 * (j + 1)],
                                         A36[:, 128 * b:128 * (b + 1)],
                                         B36[:, 512 * cj:512 * (cj + 1)],
                                         start=True, stop=True)
                    for s in range(4):
                        s8 = 4 * h + s
                        nc.vector.max(cand[:, 8 * s8:8 * (s8 + 1)],
                                      negd[:, 384 * s:384 * (s + 1)])
                        nc.vector.max_index(segloc[:, 8 * s8:8 * (s8 + 1)],
                                            cand[:, 8 * s8:8 * (s8 + 1)],
                                            negd[:, 384 * s:384 * (s + 1)])
                jc16 = small.tile([128, 64], u16, tag="jc16", bufs=2)
                nc.vector.tensor_tensor(jc16[:], segloc[:], segb64u[:], Alu.add)
                v16 = v16_all[:, b, :]
                mrc = small.tile([128, 64], f32, tag="mrc", bufs=2)
                cp16 = small.tile([128, 16], u16, tag="cp16", bufs=2)
                nc.vector.max(v16[:, 0:8], cand[:])
                nc.vector.max_index(cp16[:, 0:8], v16[:, 0:8], cand[:])
                nc.vector.match_replace(mrc[:], v16[:, 0:8], cand[:], -1e30)
                nc.vector.max(v16[:, 8:16], mrc[:])
                nc.vector.max_index(cp16[:, 8:16], v16[:, 8:16], mrc[:])
                rankmap = small.tile([128, 64], i16, tag="rankmap", bufs=2)
                nc.gpsimd.local_scatter(rankmap[:], rank16[:],
                                        cp16[:].bitcast(i16),
                                        channels=128, num_elems=64, num_idxs=16)
                nc.vector.tensor_scalar(rankmap[:], rankmap[:], 1, None,
                                        Alu.subtract)
                nc.gpsimd.local_scatter(idx_all[:, K * b:K * (b + 1)].bitcast(i16),
                                        jc16[:].bitcast(i16), rankmap[:],
                                        channels=128, num_elems=16, num_idxs=64)

                # ---------------- S3+S4 interleaved: stage idx, gather ------
                if b % BT == BT - 1:
                    bg = b // BT
                    # stage idx to DRAM wrapped layout:
                    # dram addr = (bg*K + k)*512 + (p%16)*32 + (b%BT)*8 + p//16
                    for phs in range(8):
                        nc.sync.dma_start(
                            AP(idx_dram.tensor, bg * 512 * K + phs,
                               [[32, 16], [8, BT], [512, K]]),
                            idx_all[16 * phs:16 * (phs + 1),
                                    bg * BT * K:(bg + 1) * BT * K].bitcast(i16)
                            .rearrange("p (bl k) -> p bl k", k=K))
                    for g in range(8):
                        nc.sync.dma_start(
                            idx_wrap[16 * g:16 * (g + 1), bg * K:(bg + 1) * K, :],
                            AP(idx_dram.tensor, bg * 512 * K,
                               [[32, 16], [512, K], [1, 32]]))
                    # C1: gather + transpose + fp16 fnei copies
                    s = bg
                    b0 = BT * s
                    gth = gpool.tile([128, K * BT * 128], f32, tag="gth", bufs=2)
                    nc.gpsimd.ap_gather(
                        gth[:],
                        fxT[:],
                        idx_wrap[:, s * K:(s + 1) * K, :].rearrange(
                            "p a q -> p (a q)"),
                        channels=128, num_elems=N, d=1, num_idxs=K * BT * 128)
                    for kq in range(K):
                        ptg = ps(128, BT * 67)
                        for q in range(BT):
                            nc.tensor.matmul(
                                ptg[:, 67 * q:67 * (q + 1)],
                                gth[:, kq * BT * 128 + 128 * q:
                                    kq * BT * 128 + 128 * (q + 1)],
                                ident[:, 0:67], is_transpose=True)
                        nc.scalar.copy(
                            fnei[:, b0:b0 + BT, 0:67, kq:kq + 1],
                            ptg[:].rearrange("p (b c) -> p b c", c=67)
                            .unsqueeze(3))

            # ---------------- S5 pass C2 (batched) -----------------
            ctx.close_pa()
            pc = ctx.enter_pc(tc.tile_pool(name="passc", bufs=1))
            # cos numerator: dxyz . vhat
            dxyz = pc.tile([128, NB, 3, K], f16)
            nc.vector.tensor_tensor(
                dxyz[:], fnei[:, :, 64:67, :],
                xyz16[:].unsqueeze(3).broadcast_to((128, NB, 3, K)),
                Alu.subtract)
            nc.vector.tensor_tensor(
                dxyz[:], dxyz[:],
                vhat16[:].unsqueeze(3).broadcast_to((128, NB, 3, K)),
                Alu.mult)
            nv2 = pc.tile([128, NB, K], f32)
            nc.vector.tensor_tensor(nv2[:], dxyz[:, :, 0, :], dxyz[:, :, 1, :],
                                    Alu.add)
            nc.vector.tensor_tensor(numv_all[:], nv2[:], dxyz[:, :, 2, :], Alu.add)

            # d2/dist from saved v16 (d2 = sq_i - v16)
            d2k = pc.tile([128, NB, K], f32)
            nc.vector.tensor_tensor(
                d2k[:],
                sq_p[:].broadcast_to((128, NB, K)),
                v16_all[:], Alu.subtract)
            sqd = pc.tile([128, NB, K], f32)
            nc.scalar.activation(sqd[:], d2k[:], Act.Sqrt,
                                 bias=zero128[:], scale=1.0)
            rden = pc.tile([128, NB, K], f32)
            nc.vector.tensor_scalar(rden[:], sqd[:], 1e-6, None, Alu.max)
            nc.vector.reciprocal(rden[:], rden[:])
            ek = pc.tile([128, NB, K], f32)
            nc.vector.tensor_tensor(ek[:], numv_all[:], rden[:], Alu.mult)
            nc.scalar.activation(ek[:], ek[:], Act.Exp,
                                 bias=zero128[:], scale=1.0 / TAU)
            se = pc.tile([128, NB, 1], f32)
            nc.vector.tensor_reduce(se[:], ek[:], axis=AX.X, op=Alu.add)
            rse = pc.tile([128, NB, 1], f32)
            nc.vector.reciprocal(rse[:], se[:])
            nc.vector.tensor_tensor(ek[:], ek[:],
                                    rse[:].broadcast_to((128, NB, K)), Alu.mult)
            uw16 = pc.tile([128, NB, K], f16)
            nc.vector.tensor_tensor(uw16[:], ek[:],
                                    de16[:].broadcast_to((128, NB, K)), Alu.add)
            if debug:
                uwf = pc.tile([128, NB, K], f32)
                nc.vector.tensor_copy(uwf[:], uw16[:])
                nc.sync.dma_start(AP(dbg_ext["d_uw"], 0, [[NB * K, 128], [1, NB * K]]),
                                  uwf[:])
                nc.sync.dma_start(AP(dbg_ext["d_v16"], 0, [[NB * K, 128], [1, NB * K]]),
                                  v16_all[:])
                nc.sync.dma_start(AP(dbg_ext["d_numv"], 0, [[NB * K, 128], [1, NB * K]]),
                                  numv_all[:])

            # dist stats for reaction input
            dp = small.tile([128, NB, 2], f32)
            ndsl = dp[:, :, 0:1]
            nvsl = dp[:, :, 1:2]
            nc.vector.tensor_reduce(ndsl, sqd[:], axis=AX.X, op=Alu.add)
            nc.vector.tensor_scalar(ndsl, ndsl, 1.0 / K, None, Alu.mult)
            d2m = pc.tile([128, NB, 1], f32)
            nc.vector.tensor_reduce(d2m[:], d2k[:], axis=AX.X, op=Alu.add)
            nc.vector.tensor_scalar(d2m[:], d2m[:], 1.0 / K, None, Alu.mult)
            nd2 = pc.tile([128, NB, 1], f32)
            nc.vector.tensor_tensor(nd2[:], ndsl, ndsl, Alu.mult)
            nc.vector.tensor_tensor(nvsl, d2m[:], nd2[:], Alu.subtract)

            # weighted aggregation: agg = sum_k uw16 * fnei_f   (fp16 2x tree)
            agg = big.tile([128, NB, C], f32)
            apool = ctx.enter_pc(tc.tile_pool(name="aggp", bufs=1))
            for g in range(NSL):
                b0 = BT * g
                prod = apool.tile([128, BT, C, K], f16, tag="prod")
                nc.vector.tensor_tensor(
                    prod[:], fnei[:, b0:b0 + BT, 0:64, :],
                    uw16[:, b0:b0 + BT, :].unsqueeze(2).broadcast_to(
                        (128, BT, C, K)),
                    Alu.mult)
                s1 = apool.tile([128, BT, C, 8], f16, tag="s1")
                nc.vector.tensor_tensor(s1[:], prod[:, :, :, 0:8],
                                        prod[:, :, :, 8:16], Alu.add)
                s2 = apool.tile([128, BT, C, 4], f16, tag="s2")
                nc.vector.tensor_tensor(s2[:], s1[:, :, :, 0:4],
                                        s1[:, :, :, 4:8], Alu.add)
                s3 = apool.tile([128, BT, C, 2], f16, tag="s3")
                nc.vector.tensor_tensor(s3[:], s2[:, :, :, 0:2],
                                        s2[:, :, :, 2:4], Alu.add)
                nc.vector.tensor_tensor(agg[:, b0:b0 + BT, :],
                                        s3[:, :, :, 0], s3[:, :, :, 1], Alu.add)

            if debug:
                nc.sync.dma_start(AP(dbg_ext["d_agg"], 0, [[NB * C, 128], [1, NB * C]]),
                                  agg[:])
                idxf = pc.tile([128, NB * K], f32, tag="idxf")
                nc.vector.tensor_copy(idxf[:], idx_all[:])
                nc.sync.dma_start(AP(dbg_ext["d_idx"], 0, [[NB * K, 128], [1, NB * K]]),
                                  idxf[:])
                nc.sync.dma_start(AP(dbg_ext["d_heads"], 0, [[N, 5], [1, N]]),
                                  headsT[:])

            # ---------------- S5b reaction + BN -----------------
            # dist stats transposed into headsT rows 3:5 (dpre/upre already
            # consumed); Wr1hT covers [flow(3), nd, nv] in one matmul
            late = ctx.enter_pc(tc.tile_pool(name="late", bufs=1))
            for j in range(6):
                ptd = ps(2, 512)
                for q in range(4):
                    b = 4 * j + q
                    nc.tensor.matmul(ptd[:, 128 * q:128 * (q + 1)],
                                     dp[:, b:b + 1, :], ident[:, :],
                                     is_transpose=True)
                nc.scalar.copy(headsT[3:5, 512 * j:512 * (j + 1)], ptd[:])

            x_sb = late.tile([C, N], f32)
            xs6 = small.tile([C, 6], f32)
            x2s6 = small.tile([C, 6], f32)
            scr = late.tile([C, N], f32)
            for j in range(6):
                sl = slice(512 * j, 512 * (j + 1))
                px = ps(C, 512)
                nc.tensor.matmul(px[:], Wr1fT[:], fT[:, sl], start=True, stop=False)
                nc.tensor.matmul(px[:], Wr1hT[:], headsT[:, sl],
                                 start=False, stop=True)
                nc.scalar.activation(x_sb[:, sl], px[:], Act.Copy, bias=0.0,
                                     scale=1.0, accum_out=xs6[:, j:j + 1])
                nc.scalar.activation(scr[:, sl], x_sb[:, sl], Act.Square,
                                     bias=zero128[0:C, :], scale=1.0,
                                     accum_out=x2s6[:, j:j + 1])
            bn_loc = small.tile([C, 2], f32)
            nc.vector.tensor_reduce(bn_loc[:, 0:1], xs6[:], axis=AX.X, op=Alu.add)
            nc.vector.tensor_reduce(bn_loc[:, 1:2], x2s6[:], axis=AX.X, op=Alu.add)
            bn_in = dram.tile([C, 2], f32)
            bn_out = dram.tile([C, 2], f32)
            bn_g = small.tile([C, 2], f32)
            if nocol:
                nc.vector.tensor_scalar(bn_g[:], bn_loc[:], float(NCORES), None,
                                        Alu.mult)
            else:
                nc.sync.dma_start(bn_in[:], bn_loc[:])
                nc.gpsimd.collective_compute(
                    "AllReduce", Alu.add, replica_groups=[list(range(NCORES))],
                    ins=[bn_in[:].opt()], outs=[bn_out[:].opt()])
                nc.sync.dma_start(bn_g[:], bn_out[:])
            Mtot = float(NCORES * N)
            mu = small.tile([C, 1], f32)
            nc.vector.tensor_scalar(mu[:], bn_g[:, 0:1], 1.0 / Mtot, None, Alu.mult)
            var = small.tile([C, 1], f32)
            nc.vector.tensor_scalar(var[:], bn_g[:, 1:2], 1.0 / Mtot, None, Alu.mult)
            mu2 = small.tile([C, 1], f32)
            nc.vector.tensor_tensor(mu2[:], mu[:], mu[:], Alu.mult)
            nc.vector.tensor_tensor(var[:], var[:], mu2[:], Alu.subtract)
            nc.vector.tensor_scalar(var[:], var[:], BN_EPS, None, Alu.add)
            nc.scalar.activation(var[:], var[:], Act.Sqrt,
                                 bias=zero128[0:C, :], scale=1.0)
            rstd = small.tile([C, 1], f32)
            nc.vector.reciprocal(rstd[:], var[:])
            s_vec = small.tile([C, 1], f32)
            nc.vector.tensor_tensor(s_vec[:], gamma_v[:], rstd[:], Alu.mult)
            b_vec = small.tile([C, 1], f32)
            nc.vector.tensor_tensor(b_vec[:], mu[:], s_vec[:], Alu.mult)
            nc.vector.tensor_tensor(b_vec[:], beta_v[:], b_vec[:], Alu.subtract)

            nc.scalar.activation(x_sb[:], x_sb[:], Act.Relu,
                                 bias=b_vec[:], scale=s_vec[:])
            for j in range(6):
                sl = slice(512 * j, 512 * (j + 1))
                pr = ps(C, 512)
                nc.tensor.matmul(pr[:], Wr2T[:], x_sb[:, sl], start=True, stop=True)
                nc.scalar.copy(scr[:, sl], pr[:])
            nc.vector.tensor_scalar(scr[:], scr[:], br2_v[:], None, Alu.add)

            # ---------------- S6 final combine -----------------
            # out = f*(1 - dt*(1+de)) + dt*(agg + TR + reac)
            de1 = small.tile([128, NB, 1], f32)
            nc.vector.tensor_scalar(de1[:], de[:], 1.0, None, Alu.add)
            alpha = small.tile([128, NB, 1], f32)
            nc.vector.tensor_scalar(alpha[:], de1[:], dtv[:, 0:1], None, Alu.mult)
            nc.vector.tensor_scalar(alpha[:], alpha[:], -1.0, 1.0, Alu.mult, Alu.add)
            fa = late.tile([128, NB, C], f32)
            nc.gpsimd.tensor_tensor(fa[:], f_sb[:],
                                    alpha[:].broadcast_to((128, NB, C)), Alu.mult)
            nc.gpsimd.tensor_tensor(agg[:], agg[:], TRp[:], Alu.add)
            out_sb = late.tile([128, NB, C], f32)
            for j in range(3):
                pt = ps(128, 512)
                for q in range(8):
                    b = 8 * j + q
                    nc.tensor.matmul(pt[:, C * q:C * (q + 1)],
                                     scr[:, 128 * b:128 * (b + 1)],
                                     ident[0:C, 0:C], is_transpose=True)
                nc.scalar.copy(out_sb[:, 8 * j:8 * (j + 1), :], pt[:])
            nc.vector.tensor_tensor(agg[:], agg[:], out_sb[:], Alu.add)
            nc.vector.scalar_tensor_tensor(out_sb[:], agg[:], dtv[:], fa[:],
                                           Alu.mult, Alu.add)
            nc.sync.dma_start(AP(out_ext, 0, [[C, 128], [128 * C, NB], [1, C]]),
                              out_sb[:])

    nc.compile()
    return nc


@functools.cache
def _get_nc(debug=False):
    return _build(debug=debug)


def _run(nc, inputs, trace=False):
    from concourse.bass_utils import run_bass_kernel_spmd
    f_seq = np.ascontiguousarray(np.asarray(inputs["f_seq"], dtype=np.float32))
    xyz = np.ascontiguousarray(np.asarray(inputs["xyz"], dtype=np.float32))
    in_maps = []
    for core in range(NCORES):
        b, l = divmod(core, L)
        m = {"f": f_seq[b, l], "xyz": xyz[b, l]}
        for k in WEIGHT_NAMES:
            m[k] = np.ascontiguousarray(
                np.asarray(inputs[k], dtype=np.float32).reshape(-1))
        in_maps.append(m)
    return run_bass_kernel_spmd(nc, in_maps, core_ids=list(range(NCORES)),
                                trace=trace)


def kernel(**inputs):
    nc = _get_nc()
    res = _run(nc, inputs)
    out = np.stack([np.asarray(res.results[i]["out"]) for i in range(NCORES)])
    return out.reshape(B, L, N, C).astype(np.float32)


# revision 24
# speedup vs baseline: 1.1013x; 1.1013x over previous
"""Trainium2 Bass kernel for ADRiverDynamics (gnn_message_passing).

8 independent point clouds (B*L=8), one per NeuronCore (pure data parallel),
plus one tiny AllReduce for global BatchNorm statistics.

Per-core pipeline (cloud of N=3072 points, C=64 channels, K=16 neighbors):
  S1  bf16 3-way-split tensors A36/B36 for the distance matmul (one 36-row
      bf16 matmul per 512-chunk instead of a 4-pass fp32 matmul; x = h+m+l
      with h,m,l bf16 plus a 4th augmented coordinate carrying (1, -sq_j),
      giving fp32-grade d2 accuracy); PE transposes (fxT = [f|xyz]
      channel-major); head convs; gate conv
  S2  pass A per block: negd' = 2 x_i.x_j - sq_j (PSUM, 1536-wide halves,
      Act copy to SBUF), per-row top-16 via segmented Max/MaxIndex (the
      per-row -sq_i shift is order-invariant, fixed up via v16), merge via
      match_replace + gpsimd rank trick; idx staged through DRAM into the
      wrapped gather layout; softmax pass C pipelined one slice behind:
      gather (Pool) -> PE transposes -> fp16 k-minor fnei -> cos/softmax
      (d2 from saved top-k values, no gather) -> fp16 2x weighted tree ->
      dist stats -> reaction conv chunk
  S3  global-batch BN (AllReduce) + relu + conv2, final combine
"""
import functools
import numpy as np

B, L, N, C, K = 2, 4, 3072, 64, 16
NB = N // 128          # 24 point blocks
TAU = 0.15
BN_EPS = 1e-5
NCORES = 8
BT = 4                 # blocks per gather slice
NSL = NB // BT         # gather slices

WEIGHT_NAMES = ["Wf", "bf", "Wd", "bd", "Wu", "bu", "Wg1", "bg1", "Wg2", "bg2",
                "Wgate", "bgate", "Wr1", "br1", "gamma", "beta", "Wr2", "br2",
                "log_dt"]


def _build(debug=False, nocol=False):
    import contextlib
    from concourse import bacc
    import concourse.bass as bass
    import concourse.tile as tile
    import concourse.mybir as mybir
    from concourse import masks

    f32 = mybir.dt.float32
    bf16 = mybir.dt.bfloat16
    f16 = mybir.dt.float16
    u16 = mybir.dt.uint16
    i16 = mybir.dt.int16
    Alu = mybir.AluOpType
    Act = mybir.ActivationFunctionType
    AX = mybir.AxisListType
    AP = bass.AP

    nc = bacc.Bacc("TRN2", target_bir_lowering=False, debug=False,
                   num_devices=NCORES)

    f_ext = nc.dram_tensor("f", [N, C], f32, kind="ExternalInput")
    xyz_ext = nc.dram_tensor("xyz", [N, 3], f32, kind="ExternalInput")
    wshapes = {"Wf": [3, C], "bf": [3], "Wd": [1, C], "bd": [1], "Wu": [1, C],
               "bu": [1], "Wg1": [C, 3], "bg1": [C], "Wg2": [C, C], "bg2": [C],
               "Wgate": [C, C], "bgate": [C], "Wr1": [C, C + 5], "br1": [C],
               "gamma": [C], "beta": [C], "Wr2": [C, C], "br2": [C],
               "log_dt": [1]}
    w_ext = {k: nc.dram_tensor(k, shp, f32, kind="ExternalInput")
             for k, shp in wshapes.items()}
    out_ext = nc.dram_tensor("out", [N, C], f32, kind="ExternalOutput")
    dbg_ext = {}
    if debug:
        for k, shp in {"d_idx": [128, NB * K], "d_agg": [128, NB * C],
                       "d_uw": [128, NB * K], "d_v16": [128, NB * K],
                       "d_numv": [128, NB * K], "d_heads": [5, N]}.items():
            dbg_ext[k] = nc.dram_tensor(k, shp, f32, kind="ExternalOutput")

    with tile.TileContext(nc) as tc:
        with contextlib.ExitStack() as ctx:
            cpool = ctx.enter_context(tc.tile_pool(name="consts", bufs=1))
            big = ctx.enter_context(tc.tile_pool(name="big", bufs=1))
            dram = ctx.enter_context(tc.tile_pool(name="dram", bufs=1, space="DRAM"))
            psum = ctx.enter_context(tc.tile_pool(name="psum", bufs=2, space="PSUM"))
            ngps = ctx.enter_context(tc.tile_pool(name="ngps", bufs=2, space="PSUM"))
            small = ctx.enter_context(tc.tile_pool(name="small", bufs=1))
            latex = ctx.enter_context(tc.tile_pool(name="latex", bufs=1))
            s1stk = contextlib.ExitStack()
            spl = s1stk.enter_context(tc.tile_pool(name="split", bufs=1))
            gatep = s1stk.enter_context(tc.tile_pool(name="gate", bufs=1))

            def ps(p, fr):
                return psum.tile([p, fr], f32, tag="ps", name="pst")

            # ---------------- S1a: split tensors (issued first) ----------
            xyz_sb = big.tile([128, NB, 3], f32)
            nc.sync.dma_start(xyz_sb[:], AP(xyz_ext, 0, [[3, 128], [128 * 3, NB], [1, 3]]))

            pmA = spl.tile([128, NB, 128], bf16)
            pmB = spl.tile([128, NB, 128], bf16)
            nc.gpsimd.memset(pmA[:], 0.0)
            nc.gpsimd.memset(pmB[:], 0.0)

            rt2 = float(np.sqrt(2.0))
            yv = spl.tile([128, NB, 3], f32)
            nc.vector.tensor_scalar(yv[:], xyz_sb[:], rt2, None, Alu.mult)
            x2 = spl.tile([128, NB, 3], f32)
            nc.vector.tensor_tensor(x2[:], xyz_sb[:], xyz_sb[:], Alu.mult)
            sq_p = small.tile([128, NB, 1], f32)
            nc.vector.tensor_reduce(sq_p[:], x2[:], axis=AX.X, op=Alu.add)
            nsq = spl.tile([128, NB, 1], f32)
            nc.vector.tensor_scalar(nsq[:], sq_p[:], -1.0, None, Alu.mult)

            r3 = spl.tile([128, NB, 3], f32)
            r3b = spl.tile([128, NB, 3], f32)
            rs = spl.tile([128, NB, 1], f32)
            rs2 = spl.tile([128, NB, 1], f32)
            for pm, aug in ((pmA, None), (pmB, nsq)):
                nc.vector.tensor_copy(pm[:, :, 0:3], yv[:])
                nc.vector.tensor_tensor(r3[:], yv[:], pm[:, :, 0:3], Alu.subtract)
                nc.vector.tensor_copy(pm[:, :, 4:7], r3[:])
                nc.vector.tensor_tensor(r3b[:], r3[:], pm[:, :, 4:7], Alu.subtract)
                nc.vector.tensor_copy(pm[:, :, 8:11], r3b[:])
                if aug is None:
                    nc.vector.memset(pm[:, :, 3:4], 1.0)
                else:
                    nc.vector.tensor_copy(pm[:, :, 3:4], aug[:])
                    nc.vector.tensor_tensor(rs[:], aug[:], pm[:, :, 3:4], Alu.subtract)
                    nc.vector.tensor_copy(pm[:, :, 7:8], rs[:])
                    nc.vector.tensor_tensor(rs2[:], rs[:], pm[:, :, 7:8], Alu.subtract)
                    nc.vector.tensor_copy(pm[:, :, 11:12], rs2[:])

            TA = spl.tile([128, N], bf16)
            TB = spl.tile([128, N], bf16)
            for b in range(NB):
                eng = nc.sync if b % 2 == 0 else nc.scalar
                eng.dma_start_transpose(TA[:, 128 * b:128 * (b + 1)], pmA[:, b, :])
                eng.dma_start_transpose(TB[:, 128 * b:128 * (b + 1)], pmB[:, b, :])
            # A36 rows: [Ah(4) x3, Am(4) x3, Al(4) x3]; B36: [Bh, Bm, Bl](12) x3
            A36 = big.tile([36, N], bf16)
            B36 = big.tile([36, N], bf16)
            for t in range(3):
                for p in range(3):
                    eng = nc.sync if p % 2 == 0 else nc.scalar
                    eng.dma_start(A36[12 * p + 4 * t:12 * p + 4 * t + 4, :],
                                  TA[4 * p:4 * p + 4, :])
                nc.scalar.dma_start(B36[12 * t:12 * (t + 1), :], TB[0:12, :])

            # ---------------- S1b: weights + transposes + convs ----------
            ident = cpool.tile([128, 128], f32)
            masks.make_identity(nc, ident[:])

            WhT = cpool.tile([C, 5], f32)
            nc.sync.dma_start(WhT[:, 0:3], AP(w_ext["Wf"], 0, [[1, C], [C, 3]]))
            nc.sync.dma_start(WhT[:, 3:4], AP(w_ext["Wd"], 0, [[1, C], [C, 1]]))
            nc.sync.dma_start(WhT[:, 4:5], AP(w_ext["Wu"], 0, [[1, C], [C, 1]]))
            bhead = cpool.tile([5, 1], f32)
            nc.sync.dma_start(bhead[0:3, :], AP(w_ext["bf"], 0, [[1, 3], [1, 1]]))
            nc.sync.dma_start(bhead[3:4, :], AP(w_ext["bd"], 0, [[1, 1], [1, 1]]))
            nc.sync.dma_start(bhead[4:5, :], AP(w_ext["bu"], 0, [[1, 1], [1, 1]]))

            WgateT = cpool.tile([C, C], f32)
            nc.sync.dma_start(WgateT[:], AP(w_ext["Wgate"], 0, [[1, C], [C, C]]))
            Wg1T = cpool.tile([3, C], f32)
            nc.sync.dma_start(Wg1T[:], AP(w_ext["Wg1"], 0, [[1, 3], [3, C]]))
            Wg2T = cpool.tile([C, C], f32)
            nc.sync.dma_start(Wg2T[:], AP(w_ext["Wg2"], 0, [[1, C], [C, C]]))
            Wr1fT = cpool.tile([C, C], f32)
            nc.sync.dma_start(Wr1fT[:], AP(w_ext["Wr1"], 0, [[1, C], [C + 5, C]]))
            Wr1hT = cpool.tile([5, C], f32)
            nc.sync.dma_start(Wr1hT[:], AP(w_ext["Wr1"], C, [[1, 5], [C + 5, C]]))
            Wr2T = cpool.tile([C, C], f32)
            nc.sync.dma_start(Wr2T[:], AP(w_ext["Wr2"], 0, [[1, C], [C, C]]))

            def vec_col(name):
                t = cpool.tile([C, 1], f32, tag=name, name=name + "_v")
                nc.sync.dma_start(t[:], AP(w_ext[name], 0, [[1, C], [1, 1]]))
                return t
            bgate_v = vec_col("bgate")
            bg1_v = vec_col("bg1")
            bg2_v = vec_col("bg2")
            br2_v = vec_col("br2")
            gamma_v = vec_col("gamma")
            beta_v = vec_col("beta")

            zero128 = cpool.tile([128, 1], f32)
            nc.vector.memset(zero128[:], 0.0)
            ones128 = cpool.tile([128, 1], f32)
            nc.vector.memset(ones128[:], 1.0)
            segb64u = cpool.tile([128, 64], u16)
            nc.gpsimd.iota(segb64u[:], pattern=[[384, 8], [0, 8]],
                           channel_multiplier=0)
            rank16 = cpool.tile([128, 16], i16)
            nc.gpsimd.iota(rank16[:], pattern=[[1, 16]], base=1,
                           channel_multiplier=0)
            dtv = cpool.tile([128, 1], f32)

            f_sb = big.tile([128, NB, C], f32)
            nc.sync.dma_start(f_sb[:], AP(f_ext, 0, [[C, 128], [128 * C, NB], [1, C]]))

            # fxT: rows 0:64 f, 64:67 xyz (fp32, gather source + conv input)
            fxT = big.tile([128, N], f32)
            fT = fxT[0:C, :]
            for j in range(6):
                pt = ps(C, 512)
                for q in range(4):
                    b = 4 * j + q
                    nc.tensor.matmul(pt[:, 128 * q:128 * (q + 1)],
                                     f_sb[:, b:b + 1, :], ident[:, :],
                                     is_transpose=True)
                nc.scalar.copy(fxT[0:C, 512 * j:512 * (j + 1)], pt[:])
            for j in range(6):
                pt = ps(3, 512)
                for q in range(4):
                    b = 4 * j + q
                    nc.tensor.matmul(pt[:, 128 * q:128 * (q + 1)],
                                     xyz_sb[:, b:b + 1, :], ident[:, :],
                                     is_transpose=True)
                nc.scalar.copy(fxT[C:C + 3, 512 * j:512 * (j + 1)], pt[:])

            headsT = big.tile([5, N], f32)
            gateT = gatep.tile([C, N], f32)
            for j in range(6):
                sl = slice(512 * j, 512 * (j + 1))
                ph = ps(5, 512)
                nc.tensor.matmul(ph[:], WhT[:], fT[:, sl], start=True, stop=True)
                nc.scalar.activation(headsT[:, sl], ph[:], Act.Identity,
                                     bias=bhead[:], scale=1.0)
                pg = ps(C, 512)
                nc.tensor.matmul(pg[:], WgateT[:], fT[:, sl], start=True, stop=True)
                nc.scalar.activation(gateT[:, sl], pg[:], Act.Sigmoid,
                                     bias=bgate_v[:], scale=1.0)

            hp = small.tile([128, NB, 5], f32)
            pt5 = ps(128, NB * 5)
            for b in range(NB):
                nc.tensor.matmul(pt5[:, 5 * b:5 * (b + 1)],
                                 headsT[:, 128 * b:128 * (b + 1)], ident[0:5, 0:5],
                                 is_transpose=True)
            nc.vector.tensor_copy(hp[:], pt5[:])

            flow_p = hp[:, :, 0:3]
            # de = softplus(dpre) * (1 + sigmoid(upre))
            de = small.tile([128, NB, 1], f32)
            sgu = small.tile([128, NB, 1], f32)
            nc.scalar.activation(sgu[:], hp[:, :, 4:5], Act.Sigmoid,
                                 bias=zero128[:], scale=1.0)
            nc.vector.tensor_scalar(sgu[:], sgu[:], 1.0, None, Alu.add)
            tmp_b = small.tile([128, NB, 1], f32)
            nc.scalar.activation(tmp_b[:], hp[:, :, 3:4], Act.Exp,
                                 bias=zero128[:], scale=1.0)
            nc.vector.tensor_scalar(tmp_b[:], tmp_b[:], 1.0, None, Alu.add)
            nc.scalar.activation(tmp_b[:], tmp_b[:], Act.Ln,
                                 bias=zero128[:], scale=1.0)
            nc.vector.tensor_tensor(de[:], tmp_b[:], sgu[:], Alu.mult)
            de16 = small.tile([128, NB, 1], f32)
            nc.vector.tensor_scalar(de16[:], de[:], 1.0 / K, None, Alu.mult)

            nc.sync.dma_start(dtv[:], AP(w_ext["log_dt"], 0, [[0, 128], [1, 1]]))
            nc.scalar.activation(dtv[:], dtv[:], Act.Exp, bias=zero128[:], scale=1.0)
            nc.vector.tensor_scalar(dtv[:], dtv[:], 1e-4, 10.0, Alu.max, Alu.min)

            # flow normalization
            fl2 = small.tile([128, NB, 3], f32)
            nc.vector.tensor_tensor(fl2[:], flow_p, flow_p, Alu.mult)
            vn = small.tile([128, NB, 1], f32)
            nc.vector.tensor_reduce(vn[:], fl2[:], axis=AX.X, op=Alu.add)
            nc.scalar.activation(vn[:], vn[:], Act.Sqrt, bias=zero128[:], scale=1.0)
            nc.vector.tensor_scalar(vn[:], vn[:], 1e-6, None, Alu.max)
            rv = small.tile([128, NB, 1], f32)
            nc.vector.reciprocal(rv[:], vn[:])
            vhat16 = small.tile([128, NB, 3], f16)
            nc.vector.tensor_tensor(vhat16[:], flow_p,
                                    rv[:].broadcast_to((128, NB, 3)), Alu.mult)
            xyz16 = small.tile([128, NB, 3], f16)
            nc.vector.tensor_copy(xyz16[:], xyz_sb[:])

            # global advection gate
            pfg = ps(1, NB * 5)
            nc.tensor.matmul(pfg[:], ones128[:], hp[:].rearrange("p a b -> p (a b)"),
                             start=True, stop=True)
            fgrow = small.tile([1, NB, 5], f32)
            nc.vector.tensor_copy(fgrow[:], pfg[:])
            fgm_r = small.tile([1, 5], f32)
            nc.vector.tensor_reduce(
                fgm_r[:], fgrow[:].transpose([0, 2, 1]),
                axis=AX.X, op=Alu.add)
            nc.vector.tensor_scalar(fgm_r[:], fgm_r[:], 1.0 / N, None, Alu.mult)
            pft = ps(5, 1)
            nc.tensor.matmul(pft[:], fgm_r[0:1, :], ones128[0:1, 0:1],
                             is_transpose=True)
            fgm = small.tile([5, 1], f32)
            nc.vector.tensor_copy(fgm[:], pft[:])
            pg1 = ps(C, 1)
            nc.tensor.matmul(pg1[:], Wg1T[:], fgm[0:3, :], start=True, stop=True)
            hg = small.tile([C, 1], f32)
            nc.scalar.activation(hg[:], pg1[:], Act.Relu, bias=bg1_v[:], scale=1.0)
            pg2 = ps(C, 1)
            nc.tensor.matmul(pg2[:], Wg2T[:], hg[:], start=True, stop=True)
            fgf = small.tile([C, 1], f32)
            nc.vector.tensor_scalar(fgf[:], pg2[:], bg2_v[:], None, Alu.add)
            # TR = gate * fgf (on Act), transposed to point layout
            nc.scalar.activation(gateT[:], gateT[:], Act.Identity,
                                 bias=zero128[0:C, :], scale=fgf[:])
            TRp = big.tile([128, NB, C], f32)
            for j in range(3):
                pt = ps(128, 512)
                for q in range(8):
                    b = 8 * j + q
                    nc.tensor.matmul(pt[:, C * q:C * (q + 1)],
                                     gateT[:, 128 * b:128 * (b + 1)],
                                     ident[0:C, 0:C], is_transpose=True)
                nc.scalar.copy(TRp[:, 8 * j:8 * (j + 1), :], pt[:])

            # ---------------- S2 pass A + pipelined pass C ---------------
            s1stk.close()
            loopstk = contextlib.ExitStack()
            gpool = loopstk.enter_context(tc.tile_pool(name="gth", bufs=3))
            pc = loopstk.enter_context(tc.tile_pool(name="passc", bufs=1))
            ngsb = loopstk.enter_context(tc.tile_pool(name="ngsb", bufs=2))
            idx_all = big.tile([128, NB * K], u16)
            v16_all = big.tile([128, NB, K], f32)
            NBG = NB // BT
            idx_dram = dram.tile([NBG * K * 512], i16)
            idx_wrap = big.tile([128, NBG * K, 32], i16)
            agg = big.tile([128, NB, C], f32)
            dp = small.tile([128, NB, 2], f32)
            x_sb = latex.tile([C, N], f32)
            xs6 = small.tile([C, 6], f32)
            x2s6 = small.tile([C, 6], f32)
            gth_tiles = {}

            def process_slice(s):
                """Pass C for slice s: transpose gathered cols, softmax
                aggregation, dist stats, reaction conv chunk (cols 512s)."""
                b0 = BT * s
                gth_h = gth_tiles.pop(s)
                fnei = pc.tile([128, BT, 67, K], f16, tag="fnei")
                for kq in range(K):
                    gth = gth_h[kq // 8]
                    kqh = kq % 8
                    ptg = ps(128, BT * 67)
                    for q in range(BT):
                        nc.tensor.matmul(
                            ptg[:, 67 * q:67 * (q + 1)],
                            gth[:, kqh * BT * 128 + 128 * q:
                                kqh * BT * 128 + 128 * (q + 1)],
                            ident[:, 0:67], is_transpose=True)
                    nc.scalar.copy(
                        fnei[:, :, 0:67, kq:kq + 1],
                        ptg[:].rearrange("p (b c) -> p b c", c=67).unsqueeze(3))
                # cos numerator
                dxyz = pc.tile([128, BT, 3, K], f16, tag="dxyz")
                nc.vector.tensor_tensor(
                    dxyz[:], fnei[:, :, 64:67, :],
                    xyz16[:, b0:b0 + BT, :].unsqueeze(3).broadcast_to(
                        (128, BT, 3, K)), Alu.subtract)
                nc.vector.tensor_tensor(
                    dxyz[:], dxyz[:],
                    vhat16[:, b0:b0 + BT, :].unsqueeze(3).broadcast_to(
                        (128, BT, 3, K)), Alu.mult)
                numv = pc.tile([128, BT, K], f32, tag="numv")
                nc.vector.tensor_tensor(numv[:], dxyz[:, :, 0, :],
                                        dxyz[:, :, 1, :], Alu.add)
                nc.vector.tensor_tensor(numv[:], numv[:], dxyz[:, :, 2, :],
                                        Alu.add)
                # d2/dist from saved v16
                d2k = pc.tile([128, BT, K], f32, tag="d2k")
                nc.vector.tensor_tensor(
                    d2k[:], sq_p[:, b0:b0 + BT, :].broadcast_to((128, BT, K)),
                    v16_all[:, b0:b0 + BT, :], Alu.subtract)
                sqd = pc.tile([128, BT, K], f32, tag="sqd")
                nc.scalar.activation(sqd[:], d2k[:], Act.Sqrt,
                                     bias=zero128[:], scale=1.0)
                rden = pc.tile([128, BT, K], f32, tag="rden")
                nc.vector.tensor_scalar(rden[:], sqd[:], 1e-6, None, Alu.max)
                nc.vector.reciprocal(rden[:], rden[:])
                ek = pc.tile([128, BT, K], f32, tag="ek")
                nc.vector.tensor_tensor(ek[:], numv[:], rden[:], Alu.mult)
                nc.scalar.activation(ek[:], ek[:], Act.Exp,
                                     bias=zero128[:], scale=1.0 / TAU)
                se = pc.tile([128, BT, 1], f32, tag="se")
                nc.vector.tensor_reduce(se[:], ek[:], axis=AX.X, op=Alu.add)
                rse = pc.tile([128, BT, 1], f32, tag="rse")
                nc.vector.reciprocal(rse[:], se[:])
                nc.vector.tensor_tensor(ek[:], ek[:],
                                        rse[:].broadcast_to((128, BT, K)),
                                        Alu.mult)
                uw16 = pc.tile([128, BT, K], f16, tag="uw16")
                nc.vector.tensor_tensor(
                    uw16[:], ek[:],
                    de16[:, b0:b0 + BT, :].broadcast_to((128, BT, K)), Alu.add)
                # weighted aggregation (fp16 2x tree)
                prod = pc.tile([128, BT, C, K], f16, tag="prod")
                nc.vector.tensor_tensor(
                    prod[:], fnei[:, :, 0:64, :],
                    uw16[:].unsqueeze(2).broadcast_to((128, BT, C, K)),
                    Alu.mult)
                s1t = pc.tile([128, BT, C, 8], f16, tag="s1t")
                nc.vector.tensor_tensor(s1t[:], prod[:, :, :, 0:8],
                                        prod[:, :, :, 8:16], Alu.add)
                s2t = pc.tile([128, BT, C, 4], f16, tag="s2t")
                nc.vector.tensor_tensor(s2t[:], s1t[:, :, :, 0:4],
                                        s1t[:, :, :, 4:8], Alu.add)
                s3t = pc.tile([128, BT, C, 2], f16, tag="s3t")
                nc.vector.tensor_tensor(s3t[:], s2t[:, :, :, 0:2],
                                        s2t[:, :, :, 2:4], Alu.add)
                nc.vector.tensor_tensor(agg[:, b0:b0 + BT, :],
                                        s3t[:, :, :, 0], s3t[:, :, :, 1],
                                        Alu.add)
                if debug:
                    uwf = pc.tile([128, BT, K], f32, tag="uwf")
                    nc.vector.tensor_copy(uwf[:], uw16[:])
                    nc.sync.dma_start(
                        AP(dbg_ext["d_uw"], b0 * K,
                           [[NB * K, 128], [K, BT], [1, K]]), uwf[:])
                    nc.sync.dma_start(
                        AP(dbg_ext["d_numv"], b0 * K,
                           [[NB * K, 128], [K, BT], [1, K]]), numv[:])
                # dist stats
                ndsl = dp[:, b0:b0 + BT, 0:1]
                nvsl = dp[:, b0:b0 + BT, 1:2]
                nc.vector.tensor_reduce(ndsl, sqd[:], axis=AX.X, op=Alu.add)
                nc.vector.tensor_scalar(ndsl, ndsl, 1.0 / K, None, Alu.mult)
                d2m = pc.tile([128, BT, 1], f32, tag="d2m")
                nc.vector.tensor_reduce(d2m[:], d2k[:], axis=AX.X, op=Alu.add)
                nc.vector.tensor_scalar(d2m[:], d2m[:], 1.0 / K, None, Alu.mult)
                nd2 = pc.tile([128, BT, 1], f32, tag="nd2")
                nc.vector.tensor_tensor(nd2[:], ndsl, ndsl, Alu.mult)
                nc.vector.tensor_tensor(nvsl, d2m[:], nd2[:], Alu.subtract)
                # dist stats -> headsT rows 3:5 (dpre/upre already consumed)
                sl = slice(512 * s, 512 * (s + 1))
                ptd = ps(2, 512)
                for q in range(BT):
                    nc.tensor.matmul(ptd[:, 128 * q:128 * (q + 1)],
                                     dp[:, b0 + q:b0 + q + 1, :], ident[:, :],
                                     is_transpose=True)
                nc.scalar.copy(headsT[3:5, sl], ptd[:])
                # reaction conv chunk
                px = ps(C, 512)
                nc.tensor.matmul(px[:], Wr1fT[:], fT[:, sl], start=True,
                                 stop=False)
                nc.tensor.matmul(px[:], Wr1hT[:], headsT[:, sl],
                                 start=False, stop=True)
                nc.scalar.activation(x_sb[:, sl], px[:], Act.Copy, bias=0.0,
                                     scale=1.0, accum_out=xs6[:, s:s + 1])
                sqscr = pc.tile([C, 512], f32, tag="sqscr")
                nc.scalar.activation(sqscr[:], x_sb[:, sl], Act.Square,
                                     bias=zero128[0:C, :], scale=1.0,
                                     accum_out=x2s6[:, s:s + 1])

            for b in range(NB):
                cand = small.tile([128, 64], f32, tag="cand", bufs=2)
                segloc = small.tile([128, 64], u16, tag="segloc", bufs=2)
                for h in range(2):
                    ngp = ngps.tile([128, 1536], f32, tag="ngp")
                    for j in range(3):
                        cj = 3 * h + j
                        nc.tensor.matmul(ngp[:, 512 * j:512 * (j + 1)],
                                         A36[:, 128 * b:128 * (b + 1)],
                                         B36[:, 512 * cj:512 * (cj + 1)],
                                         start=True, stop=True)
                    negd = ngsb.tile([128, 1536], f32, tag="negd")
                    nc.scalar.copy(negd[:], ngp[:])
                    for s in range(4):
                        s8 = 4 * h + s
                        nc.vector.max(cand[:, 8 * s8:8 * (s8 + 1)],
                                      negd[:, 384 * s:384 * (s + 1)])
                        nc.vector.max_index(segloc[:, 8 * s8:8 * (s8 + 1)],
                                            cand[:, 8 * s8:8 * (s8 + 1)],
                                            negd[:, 384 * s:384 * (s + 1)])
                jc16 = small.tile([128, 64], u16, tag="jc16", bufs=2)
                nc.vector.tensor_tensor(jc16[:], segloc[:], segb64u[:], Alu.add)
                v16 = v16_all[:, b, :]
                mrc = small.tile([128, 64], f32, tag="mrc", bufs=2)
                cp16 = small.tile([128, 16], u16, tag="cp16", bufs=2)
                nc.vector.max(v16[:, 0:8], cand[:])
                nc.vector.max_index(cp16[:, 0:8], v16[:, 0:8], cand[:])
                nc.vector.match_replace(mrc[:], v16[:, 0:8], cand[:], -1e30)
                nc.vector.max(v16[:, 8:16], mrc[:])
                nc.vector.max_index(cp16[:, 8:16], v16[:, 8:16], mrc[:])
                rankmap = small.tile([128, 64], i16, tag="rankmap", bufs=2)
                nc.gpsimd.local_scatter(rankmap[:], rank16[:],
                                        cp16[:].bitcast(i16),
                                        channels=128, num_elems=64, num_idxs=16)
                nc.vector.tensor_scalar(rankmap[:], rankmap[:], 1, None,
                                        Alu.subtract)
                nc.gpsimd.local_scatter(idx_all[:, K * b:K * (b + 1)].bitcast(i16),
                                        jc16[:].bitcast(i16), rankmap[:],
                                        channels=128, num_elems=16, num_idxs=64)

                if b % BT == BT - 1:
                    bg = b // BT
                    # stage idx to DRAM wrapped layout, read back, gather
                    for phs in range(8):
                        nc.sync.dma_start(
                            AP(idx_dram.tensor, bg * 512 * K + phs,
                               [[32, 16], [8, BT], [512, K]]),
                            idx_all[16 * phs:16 * (phs + 1),
                                    bg * BT * K:(bg + 1) * BT * K].bitcast(i16)
                            .rearrange("p (bl k) -> p bl k", k=K))
                    for g in range(8):
                        nc.sync.dma_start(
                            idx_wrap[16 * g:16 * (g + 1), bg * K:(bg + 1) * K, :],
                            AP(idx_dram.tensor, bg * 512 * K,
                               [[32, 16], [512, K], [1, 32]]))
                    gth_h = []
                    for hh in range(2):
                        gthh = gpool.tile([128, 8 * BT * 128], f32, tag="gth")
                        gth_h.append(gthh)
                        nc.gpsimd.ap_gather(
                            gthh[:],
                            fxT[:],
                            idx_wrap[:, bg * K + 8 * hh:bg * K + 8 * (hh + 1),
                                     :].rearrange("p a q -> p (a q)"),
                            channels=128, num_elems=N, d=1,
                            num_idxs=8 * BT * 128)
                    gth_tiles[bg] = gth_h
                    if bg >= 1:
                        process_slice(bg - 1)
            process_slice(NSL - 1)

            if debug:
                nc.sync.dma_start(AP(dbg_ext["d_agg"], 0, [[NB * C, 128], [1, NB * C]]),
                                  agg[:])
                nc.sync.dma_start(AP(dbg_ext["d_v16"], 0, [[NB * K, 128], [1, NB * K]]),
                                  v16_all[:])
                idxf = latex.tile([128, NB * K], f32, tag="idxf")
                nc.vector.tensor_copy(idxf[:], idx_all[:])
                nc.sync.dma_start(AP(dbg_ext["d_idx"], 0, [[NB * K, 128], [1, NB * K]]),
                                  idxf[:])
                nc.sync.dma_start(AP(dbg_ext["d_heads"], 0, [[N, 5], [1, N]]),
                                  headsT[:])

            # ---------------- S3 BN + reaction tail + combine ------------
            loopstk.close()
            late = ctx.enter_context(tc.tile_pool(name="late", bufs=1))
            scr = late.tile([C, N], f32)
            bn_loc = small.tile([C, 2], f32)
            nc.vector.tensor_reduce(bn_loc[:, 0:1], xs6[:], axis=AX.X, op=Alu.add)
            nc.vector.tensor_reduce(bn_loc[:, 1:2], x2s6[:], axis=AX.X, op=Alu.add)
            bn_in = dram.tile([C, 2], f32)
            bn_out = dram.tile([C, 2], f32)
            bn_g = small.tile([C, 2], f32)
            if nocol:
                nc.vector.tensor_scalar(bn_g[:], bn_loc[:], float(NCORES), None,
                                        Alu.mult)
            else:
                nc.sync.dma_start(bn_in[:], bn_loc[:])
                nc.gpsimd.collective_compute(
                    "AllReduce", Alu.add, replica_groups=[list(range(NCORES))],
                    ins=[bn_in[:].opt()], outs=[bn_out[:].opt()])
                nc.sync.dma_start(bn_g[:], bn_out[:])
            Mtot = float(NCORES * N)
            mu = small.tile([C, 1], f32)
            nc.vector.tensor_scalar(mu[:], bn_g[:, 0:1], 1.0 / Mtot, None, Alu.mult)
            var = small.tile([C, 1], f32)
            nc.vector.tensor_scalar(var[:], bn_g[:, 1:2], 1.0 / Mtot, None, Alu.mult)
            mu2 = small.tile([C, 1], f32)
            nc.vector.tensor_tensor(mu2[:], mu[:], mu[:], Alu.mult)
            nc.vector.tensor_tensor(var[:], var[:], mu2[:], Alu.subtract)
            nc.vector.tensor_scalar(var[:], var[:], BN_EPS, None, Alu.add)
            nc.scalar.activation(var[:], var[:], Act.Sqrt,
                                 bias=zero128[0:C, :], scale=1.0)
            rstd = small.tile([C, 1], f32)
            nc.vector.reciprocal(rstd[:], var[:])
            s_vec = small.tile([C, 1], f32)
            nc.vector.tensor_tensor(s_vec[:], gamma_v[:], rstd[:], Alu.mult)
            b_vec = small.tile([C, 1], f32)
            nc.vector.tensor_tensor(b_vec[:], mu[:], s_vec[:], Alu.mult)
            nc.vector.tensor_tensor(b_vec[:], beta_v[:], b_vec[:], Alu.subtract)

            nc.scalar.activation(x_sb[:], x_sb[:], Act.Relu,
                                 bias=b_vec[:], scale=s_vec[:])
            for j in range(6):
                sl = slice(512 * j, 512 * (j + 1))
                pr = ps(C, 512)
                nc.tensor.matmul(pr[:], Wr2T[:], x_sb[:, sl], start=True, stop=True)
                nc.scalar.copy(scr[:, sl], pr[:])
            nc.vector.tensor_scalar(scr[:], scr[:], br2_v[:], None, Alu.add)

            # out = f*(1 - dt*(1+de)) + dt*(agg + TR + reac)
            de1 = small.tile([128, NB, 1], f32)
            nc.vector.tensor_scalar(de1[:], de[:], 1.0, None, Alu.add)
            alpha = small.tile([128, NB, 1], f32)
            nc.vector.tensor_scalar(alpha[:], de1[:], dtv[:, 0:1], None, Alu.mult)
            nc.vector.tensor_scalar(alpha[:], alpha[:], -1.0, 1.0, Alu.mult, Alu.add)
            fa = late.tile([128, NB, C], f32)
            nc.gpsimd.tensor_tensor(fa[:], f_sb[:],
                                    alpha[:].broadcast_to((128, NB, C)), Alu.mult)
            nc.gpsimd.tensor_tensor(agg[:], agg[:], TRp[:], Alu.add)
            out_sb = late.tile([128, NB, C], f32)
            for j in range(3):
                pt = ps(128, 512)
                for q in range(8):
                    b = 8 * j + q
                    nc.tensor.matmul(pt[:, C * q:C * (q + 1)],
                                     scr[:, 128 * b:128 * (b + 1)],
                                     ident[0:C, 0:C], is_transpose=True)
                nc.scalar.copy(out_sb[:, 8 * j:8 * (j + 1), :], pt[:])
            nc.vector.tensor_tensor(agg[:], agg[:], out_sb[:], Alu.add)
            nc.vector.scalar_tensor_tensor(out_sb[:], agg[:], dtv[:], fa[:],
                                           Alu.mult, Alu.add)
            nc.sync.dma_start(AP(out_ext, 0, [[C, 128], [128 * C, NB], [1, C]]),
                              out_sb[:])

    nc.compile()
    return nc


@functools.cache
def _get_nc(debug=False):
    return _build(debug=debug)


def _run(nc, inputs, trace=False):
    from concourse.bass_utils import run_bass_kernel_spmd
    f_seq = np.ascontiguousarray(np.asarray(inputs["f_seq"], dtype=np.float32))
    xyz = np.ascontiguousarray(np.asarray(inputs["xyz"], dtype=np.float32))
    in_maps = []
    for core in range(NCORES):
        b, l = divmod(core, L)
        m = {"f": f_seq[b, l], "xyz": xyz[b, l]}
        for k in WEIGHT_NAMES:
            m[k] = np.ascontiguousarray(
                np.asarray(inputs[k], dtype=np.float32).reshape(-1))
        in_maps.append(m)
    return run_bass_kernel_spmd(nc, in_maps, core_ids=list(range(NCORES)),
                                trace=trace)


def kernel(**inputs):
    nc = _get_nc()
    res = _run(nc, inputs)
    out = np.stack([np.asarray(res.results[i]["out"]) for i in range(NCORES)])
    return out.reshape(B, L, N, C).astype(np.float32)


# revision 32
# speedup vs baseline: 1.1827x; 1.0740x over previous
"""Trainium2 Bass kernel for ADRiverDynamics (gnn_message_passing).

8 independent point clouds (B*L=8), one per NeuronCore (pure data parallel),
plus one tiny AllReduce for global BatchNorm statistics.

Per-core pipeline (cloud of N=3072 points, C=64 channels, K=16 neighbors):
  S1  bf16 3-way-split tensors A36/B36 for the distance matmul (one 36-row
      bf16 matmul per 512-chunk instead of a 4-pass fp32 matmul; x = h+m+l
      with h,m,l bf16 plus a 4th augmented coordinate carrying (1, -sq_j),
      giving fp32-grade d2 accuracy); PE transposes (fxT = [f|xyz]
      channel-major); head convs; gate conv
  S2  pass A per block: negd' = 2 x_i.x_j - sq_j (PSUM, 1536-wide halves,
      Act copy to SBUF), per-row top-16 via segmented Max/MaxIndex (the
      per-row -sq_i shift is order-invariant, fixed up via v16), merge via
      match_replace + gpsimd rank trick; idx staged through DRAM into the
      wrapped gather layout; softmax pass C pipelined one slice behind:
      gather (Pool) -> PE transposes -> fp16 k-minor fnei -> cos/softmax
      (d2 from saved top-k values, no gather) -> fp16 2x weighted tree ->
      dist stats -> reaction conv chunk
  S3  global-batch BN (AllReduce) + relu + conv2, final combine
"""
import functools
import numpy as np

B, L, N, C, K = 2, 4, 3072, 64, 16
NB = N // 128          # 24 point blocks
TAU = 0.15
BN_EPS = 1e-5
NCORES = 8
BT = 4                 # blocks per gather slice
NSL = NB // BT         # gather slices

WEIGHT_NAMES = ["Wf", "bf", "Wd", "bd", "Wu", "bu", "Wg1", "bg1", "Wg2", "bg2",
                "Wgate", "bgate", "Wr1", "br1", "gamma", "beta", "Wr2", "br2",
                "log_dt"]


def _build(debug=False, nocol=False):
    import contextlib
    from concourse import bacc
    import concourse.bass as bass
    import concourse.tile as tile
    import concourse.mybir as mybir
    from concourse import masks

    f32 = mybir.dt.float32
    bf16 = mybir.dt.bfloat16
    f16 = mybir.dt.float16
    u16 = mybir.dt.uint16
    i16 = mybir.dt.int16
    Alu = mybir.AluOpType
    Act = mybir.ActivationFunctionType
    AX = mybir.AxisListType
    AP = bass.AP

    nc = bacc.Bacc("TRN2", target_bir_lowering=False, debug=False,
                   num_devices=NCORES)

    f_ext = nc.dram_tensor("f", [N, C], f32, kind="ExternalInput")
    xyz_ext = nc.dram_tensor("xyz", [N, 3], f32, kind="ExternalInput")
    wshapes = {"Wf": [3, C], "bf": [3], "Wd": [1, C], "bd": [1], "Wu": [1, C],
               "bu": [1], "Wg1": [C, 3], "bg1": [C], "Wg2": [C, C], "bg2": [C],
               "Wgate": [C, C], "bgate": [C], "Wr1": [C, C + 5], "br1": [C],
               "gamma": [C], "beta": [C], "Wr2": [C, C], "br2": [C],
               "log_dt": [1]}
    w_ext = {k: nc.dram_tensor(k, shp, f32, kind="ExternalInput")
             for k, shp in wshapes.items()}
    out_ext = nc.dram_tensor("out", [N, C], f32, kind="ExternalOutput")
    dbg_ext = {}
    if debug:
        for k, shp in {"d_idx": [128, NB * K], "d_agg": [128, NB * C],
                       "d_uw": [128, NB * K], "d_v16": [128, NB * K],
                       "d_numv": [128, NB * K], "d_heads": [5, N]}.items():
            dbg_ext[k] = nc.dram_tensor(k, shp, f32, kind="ExternalOutput")

    with tile.TileContext(nc) as tc:
        with contextlib.ExitStack() as ctx:
            cpool = ctx.enter_context(tc.tile_pool(name="consts", bufs=1))
            big = ctx.enter_context(tc.tile_pool(name="big", bufs=1))
            dram = ctx.enter_context(tc.tile_pool(name="dram", bufs=1, space="DRAM"))
            psum = ctx.enter_context(tc.tile_pool(name="psum", bufs=2, space="PSUM"))
            ngps = ctx.enter_context(tc.tile_pool(name="ngps", bufs=1, space="PSUM"))
            small = ctx.enter_context(tc.tile_pool(name="small", bufs=1))
            latex = ctx.enter_context(tc.tile_pool(name="latex", bufs=1))
            s1stk = contextlib.ExitStack()
            spl = s1stk.enter_context(tc.tile_pool(name="split", bufs=1))
            ptTp = s1stk.enter_context(tc.tile_pool(name="ptTp", bufs=1, space="PSUM"))
            gatep = s1stk.enter_context(tc.tile_pool(name="gate", bufs=1))

            def ps(p, fr):
                return psum.tile([p, fr], f32, tag="ps", name="pst")

            # ---------------- S1a: split tensors (issued first) ----------
            xyz_sb = big.tile([128, NB, 3], f32)
            nc.sync.dma_start(xyz_sb[:], AP(xyz_ext, 0, [[3, 128], [128 * 3, NB], [1, 3]]))

            pmA = spl.tile([128, NB, 12], bf16)
            pmB = spl.tile([128, NB, 12], bf16)

            rt2 = float(np.sqrt(2.0))
            yv = spl.tile([128, NB, 3], f32)
            nc.vector.tensor_scalar(yv[:], xyz_sb[:], rt2, None, Alu.mult)
            x2 = spl.tile([128, NB, 3], f32)
            nc.vector.tensor_tensor(x2[:], xyz_sb[:], xyz_sb[:], Alu.mult)
            sq_p = small.tile([128, NB, 1], f32)
            nc.vector.tensor_reduce(sq_p[:], x2[:], axis=AX.X, op=Alu.add)
            nsq = spl.tile([128, NB, 1], f32)
            nc.vector.tensor_scalar(nsq[:], sq_p[:], -1.0, None, Alu.mult)

            r3 = spl.tile([128, NB, 3], f32)
            r3b = spl.tile([128, NB, 3], f32)
            rs = spl.tile([128, NB, 1], f32)
            rs2 = spl.tile([128, NB, 1], f32)
            for pm, aug in ((pmA, None), (pmB, nsq)):
                nc.vector.tensor_copy(pm[:, :, 0:3], yv[:])
                nc.vector.tensor_tensor(r3[:], yv[:], pm[:, :, 0:3], Alu.subtract)
                nc.vector.tensor_copy(pm[:, :, 4:7], r3[:])
                nc.vector.tensor_tensor(r3b[:], r3[:], pm[:, :, 4:7], Alu.subtract)
                nc.vector.tensor_copy(pm[:, :, 8:11], r3b[:])
                if aug is None:
                    nc.vector.memset(pm[:, :, 3:4], 1.0)
                else:
                    nc.vector.tensor_copy(pm[:, :, 3:4], aug[:])
                    nc.vector.tensor_tensor(rs[:], aug[:], pm[:, :, 3:4], Alu.subtract)
                    nc.vector.tensor_copy(pm[:, :, 7:8], rs[:])
                    nc.vector.tensor_tensor(rs2[:], rs[:], pm[:, :, 7:8], Alu.subtract)
                    nc.vector.tensor_copy(pm[:, :, 11:12], rs2[:])

            ident16 = spl.tile([128, 128], bf16)
            masks.make_identity(nc, ident16[:])
            TA = spl.tile([12, N], bf16)
            TB = spl.tile([12, N], bf16)
            for src_pm, dst in ((pmA, TA), (pmB, TB)):
                for j in range(6):
                    ptT = ptTp.tile([12, 512], bf16, tag="ptT", name="ptT")
                    for q in range(4):
                        b = 4 * j + q
                        nc.tensor.matmul(ptT[:, 128 * q:128 * (q + 1)],
                                         src_pm[:, b, :], ident16[:, :],
                                         is_transpose=True)
                    nc.scalar.copy(dst[:, 512 * j:512 * (j + 1)], ptT[:])
            # A36 rows: [Ah(4) x3, Am(4) x3, Al(4) x3]; B36: [Bh, Bm, Bl](12) x3
            A36 = big.tile([36, N], bf16)
            B36 = big.tile([36, N], bf16)
            for t in range(3):
                for p in range(3):
                    eng = nc.sync if p % 2 == 0 else nc.scalar
                    eng.dma_start(A36[12 * p + 4 * t:12 * p + 4 * t + 4, :],
                                  TA[4 * p:4 * p + 4, :])
                nc.scalar.dma_start(B36[12 * t:12 * (t + 1), :], TB[0:12, :])

            # ---------------- S1b: weights + transposes + convs ----------
            ident = cpool.tile([128, 128], f32)
            masks.make_identity(nc, ident[:])

            WhT = cpool.tile([C, 5], f32)
            nc.sync.dma_start(WhT[:, 0:3], AP(w_ext["Wf"], 0, [[1, C], [C, 3]]))
            nc.sync.dma_start(WhT[:, 3:4], AP(w_ext["Wd"], 0, [[1, C], [C, 1]]))
            nc.sync.dma_start(WhT[:, 4:5], AP(w_ext["Wu"], 0, [[1, C], [C, 1]]))
            bhead = cpool.tile([5, 1], f32)
            nc.sync.dma_start(bhead[0:3, :], AP(w_ext["bf"], 0, [[1, 3], [1, 1]]))
            nc.sync.dma_start(bhead[3:4, :], AP(w_ext["bd"], 0, [[1, 1], [1, 1]]))
            nc.sync.dma_start(bhead[4:5, :], AP(w_ext["bu"], 0, [[1, 1], [1, 1]]))

            WgateT = cpool.tile([C, C], f32)
            nc.sync.dma_start(WgateT[:], AP(w_ext["Wgate"], 0, [[1, C], [C, C]]))
            Wg1T = cpool.tile([3, C], f32)
            nc.sync.dma_start(Wg1T[:], AP(w_ext["Wg1"], 0, [[1, 3], [3, C]]))
            Wg2T = cpool.tile([C, C], f32)
            nc.sync.dma_start(Wg2T[:], AP(w_ext["Wg2"], 0, [[1, C], [C, C]]))
            Wr1fT = cpool.tile([C, C], f32)
            nc.sync.dma_start(Wr1fT[:], AP(w_ext["Wr1"], 0, [[1, C], [C + 5, C]]))
            Wr1hT = cpool.tile([5, C], f32)
            nc.sync.dma_start(Wr1hT[:], AP(w_ext["Wr1"], C, [[1, 5], [C + 5, C]]))
            Wr2T = cpool.tile([C, C], f32)
            nc.sync.dma_start(Wr2T[:], AP(w_ext["Wr2"], 0, [[1, C], [C, C]]))

            def vec_col(name):
                t = cpool.tile([C, 1], f32, tag=name, name=name + "_v")
                nc.sync.dma_start(t[:], AP(w_ext[name], 0, [[1, C], [1, 1]]))
                return t
            bgate_v = vec_col("bgate")
            bg1_v = vec_col("bg1")
            bg2_v = vec_col("bg2")
            br2_v = vec_col("br2")
            gamma_v = vec_col("gamma")
            beta_v = vec_col("beta")

            zero128 = cpool.tile([128, 1], f32)
            nc.vector.memset(zero128[:], 0.0)
            ones128 = cpool.tile([128, 1], f32)
            nc.vector.memset(ones128[:], 1.0)
            segb64u = cpool.tile([128, 64], u16)
            nc.gpsimd.iota(segb64u[:], pattern=[[384, 8], [0, 8]],
                           channel_multiplier=0)
            rank16 = cpool.tile([128, 16], i16)
            nc.gpsimd.iota(rank16[:], pattern=[[1, 16]], base=1,
                           channel_multiplier=0)
            dtv = cpool.tile([128, 1], f32)

            f_sb = big.tile([128, NB, C], f32)
            nc.sync.dma_start(f_sb[:], AP(f_ext, 0, [[C, 128], [128 * C, NB], [1, C]]))

            # fxT: rows 0:64 f, 64:67 xyz (fp32, gather source + conv input)
            fxT = big.tile([128, N], f32)
            fT = fxT[0:C, :]
            for j in range(6):
                pt = ps(C, 512)
                for q in range(4):
                    b = 4 * j + q
                    nc.tensor.matmul(pt[:, 128 * q:128 * (q + 1)],
                                     f_sb[:, b:b + 1, :], ident[:, :],
                                     is_transpose=True)
                nc.scalar.copy(fxT[0:C, 512 * j:512 * (j + 1)], pt[:])
            for j in range(6):
                pt = ps(3, 512)
                for q in range(4):
                    b = 4 * j + q
                    nc.tensor.matmul(pt[:, 128 * q:128 * (q + 1)],
                                     xyz_sb[:, b:b + 1, :], ident[:, :],
                                     is_transpose=True)
                nc.scalar.copy(fxT[C:C + 3, 512 * j:512 * (j + 1)], pt[:])

            headsT = big.tile([5, N], f32)
            gateT = gatep.tile([C, N], f32)
            for j in range(6):
                sl = slice(512 * j, 512 * (j + 1))
                ph = ps(5, 512)
                nc.tensor.matmul(ph[:], WhT[:], fT[:, sl], start=True, stop=True)
                nc.scalar.activation(headsT[:, sl], ph[:], Act.Identity,
                                     bias=bhead[:], scale=1.0)
                pg = ps(C, 512)
                nc.tensor.matmul(pg[:], WgateT[:], fT[:, sl], start=True, stop=True)
                nc.scalar.activation(gateT[:, sl], pg[:], Act.Sigmoid,
                                     bias=bgate_v[:], scale=1.0)

            hp = small.tile([128, NB, 5], f32)
            pt5 = ps(128, NB * 5)
            for b in range(NB):
                nc.tensor.matmul(pt5[:, 5 * b:5 * (b + 1)],
                                 headsT[:, 128 * b:128 * (b + 1)], ident[0:5, 0:5],
                                 is_transpose=True)
            nc.vector.tensor_copy(hp[:], pt5[:])

            flow_p = hp[:, :, 0:3]
            # de = softplus(dpre) * (1 + sigmoid(upre))
            de = small.tile([128, NB, 1], f32)
            sgu = small.tile([128, NB, 1], f32)
            nc.scalar.activation(sgu[:], hp[:, :, 4:5], Act.Sigmoid,
                                 bias=zero128[:], scale=1.0)
            nc.vector.tensor_scalar(sgu[:], sgu[:], 1.0, None, Alu.add)
            tmp_b = small.tile([128, NB, 1], f32)
            nc.scalar.activation(tmp_b[:], hp[:, :, 3:4], Act.Exp,
                                 bias=zero128[:], scale=1.0)
            nc.vector.tensor_scalar(tmp_b[:], tmp_b[:], 1.0, None, Alu.add)
            nc.scalar.activation(tmp_b[:], tmp_b[:], Act.Ln,
                                 bias=zero128[:], scale=1.0)
            nc.vector.tensor_tensor(de[:], tmp_b[:], sgu[:], Alu.mult)
            de16 = small.tile([128, NB, 1], f32)
            nc.vector.tensor_scalar(de16[:], de[:], 1.0 / K, None, Alu.mult)

            nc.sync.dma_start(dtv[:], AP(w_ext["log_dt"], 0, [[0, 128], [1, 1]]))
            nc.scalar.activation(dtv[:], dtv[:], Act.Exp, bias=zero128[:], scale=1.0)
            nc.vector.tensor_scalar(dtv[:], dtv[:], 1e-4, 10.0, Alu.max, Alu.min)

            # flow normalization
            fl2 = small.tile([128, NB, 3], f32)
            nc.vector.tensor_tensor(fl2[:], flow_p, flow_p, Alu.mult)
            vn = small.tile([128, NB, 1], f32)
            nc.vector.tensor_reduce(vn[:], fl2[:], axis=AX.X, op=Alu.add)
            nc.scalar.activation(vn[:], vn[:], Act.Sqrt, bias=zero128[:], scale=1.0)
            nc.vector.tensor_scalar(vn[:], vn[:], 1e-6, None, Alu.max)
            rv = small.tile([128, NB, 1], f32)
            nc.vector.reciprocal(rv[:], vn[:])
            vhat16 = small.tile([128, NB, 3], f16)
            nc.vector.tensor_tensor(vhat16[:], flow_p,
                                    rv[:].broadcast_to((128, NB, 3)), Alu.mult)
            xyz16 = small.tile([128, NB, 3], f16)
            nc.vector.tensor_copy(xyz16[:], xyz_sb[:])

            # global advection gate
            pfg = ps(1, NB * 5)
            nc.tensor.matmul(pfg[:], ones128[:], hp[:].rearrange("p a b -> p (a b)"),
                             start=True, stop=True)
            fgrow = small.tile([1, NB, 5], f32)
            nc.vector.tensor_copy(fgrow[:], pfg[:])
            fgm_r = small.tile([1, 5], f32)
            nc.vector.tensor_reduce(
                fgm_r[:], fgrow[:].transpose([0, 2, 1]),
                axis=AX.X, op=Alu.add)
            nc.vector.tensor_scalar(fgm_r[:], fgm_r[:], 1.0 / N, None, Alu.mult)
            pft = ps(5, 1)
            nc.tensor.matmul(pft[:], fgm_r[0:1, :], ones128[0:1, 0:1],
                             is_transpose=True)
            fgm = small.tile([5, 1], f32)
            nc.vector.tensor_copy(fgm[:], pft[:])
            pg1 = ps(C, 1)
            nc.tensor.matmul(pg1[:], Wg1T[:], fgm[0:3, :], start=True, stop=True)
            hg = small.tile([C, 1], f32)
            nc.scalar.activation(hg[:], pg1[:], Act.Relu, bias=bg1_v[:], scale=1.0)
            pg2 = ps(C, 1)
            nc.tensor.matmul(pg2[:], Wg2T[:], hg[:], start=True, stop=True)
            fgf = small.tile([C, 1], f32)
            nc.vector.tensor_scalar(fgf[:], pg2[:], bg2_v[:], None, Alu.add)
            # TR = gate * fgf (on Act), transposed to point layout
            nc.scalar.activation(gateT[:], gateT[:], Act.Identity,
                                 bias=zero128[0:C, :], scale=fgf[:])
            TRp = big.tile([128, NB, C], f32)
            for j in range(3):
                pt = ps(128, 512)
                for q in range(8):
                    b = 8 * j + q
                    nc.tensor.matmul(pt[:, C * q:C * (q + 1)],
                                     gateT[:, 128 * b:128 * (b + 1)],
                                     ident[0:C, 0:C], is_transpose=True)
                nc.scalar.copy(TRp[:, 8 * j:8 * (j + 1), :], pt[:])

            # ---------------- S2 pass A + pipelined pass C ---------------
            s1stk.close()
            loopstk = contextlib.ExitStack()
            gpool = loopstk.enter_context(tc.tile_pool(name="gth", bufs=3))
            pc = loopstk.enter_context(tc.tile_pool(name="passc", bufs=1))
            ngsb = loopstk.enter_context(tc.tile_pool(name="ngsb", bufs=2))
            idx_all = big.tile([128, NB * K], u16)
            v16_all = big.tile([128, NB, K], f32)
            NBG = NB // BT
            idx_dram = dram.tile([NBG * K * 512], i16)
            idx_wrap = big.tile([128, NBG * K, 32], i16)
            agg = big.tile([128, NB, C], f32)
            dp = small.tile([128, NB, 2], f32)
            x_sb = latex.tile([C, N], f32)
            xs6 = small.tile([C, 6], f32)
            x2s6 = small.tile([C, 6], f32)
            gth_tiles = {}

            def process_slice(s):
                """Pass C for slice s: transpose gathered cols, softmax
                aggregation, dist stats, reaction conv chunk (cols 512s)."""
                b0 = BT * s
                gth_h = gth_tiles.pop(s)
                fnei = pc.tile([128, BT, 67, K], f16, tag="fnei")
                for kq2 in range(K // 2):
                    ptg = ps(128, 2 * BT * 67)
                    for k2 in range(2):
                        kq = 2 * kq2 + k2
                        gth = gth_h[kq // 8]
                        kqh = kq % 8
                        for q in range(BT):
                            nc.tensor.matmul(
                                ptg[:, (k2 * BT + q) * 67:(k2 * BT + q + 1) * 67],
                                gth[:, kqh * BT * 128 + 128 * q:
                                    kqh * BT * 128 + 128 * (q + 1)],
                                ident[:, 0:67], is_transpose=True)
                    nc.scalar.copy(
                        fnei[:, :, 0:67, 2 * kq2:2 * kq2 + 2],
                        ptg[:].rearrange("p (k b c) -> p b c k", k=2, c=67))
                # cos numerator
                dxyz = pc.tile([128, BT, 3, K], f16, tag="dxyz")
                nc.vector.tensor_tensor(
                    dxyz[:], fnei[:, :, 64:67, :],
                    xyz16[:, b0:b0 + BT, :].unsqueeze(3).broadcast_to(
                        (128, BT, 3, K)), Alu.subtract)
                nc.vector.tensor_tensor(
                    dxyz[:], dxyz[:],
                    vhat16[:, b0:b0 + BT, :].unsqueeze(3).broadcast_to(
                        (128, BT, 3, K)), Alu.mult)
                numv = pc.tile([128, BT, K], f32, tag="numv")
                nc.vector.tensor_tensor(numv[:], dxyz[:, :, 0, :],
                                        dxyz[:, :, 1, :], Alu.add)
                nc.vector.tensor_tensor(numv[:], numv[:], dxyz[:, :, 2, :],
                                        Alu.add)
                # d2/dist from saved v16
                d2k = pc.tile([128, BT, K], f32, tag="d2k")
                nc.vector.tensor_tensor(
                    d2k[:], sq_p[:, b0:b0 + BT, :].broadcast_to((128, BT, K)),
                    v16_all[:, b0:b0 + BT, :], Alu.subtract)
                sqd = pc.tile([128, BT, K], f32, tag="sqd")
                nc.scalar.activation(sqd[:], d2k[:], Act.Sqrt,
                                     bias=zero128[:], scale=1.0)
                rden = pc.tile([128, BT, K], f32, tag="rden")
                nc.vector.tensor_scalar(rden[:], sqd[:], 1e-6, None, Alu.max)
                nc.vector.reciprocal(rden[:], rden[:])
                ek = pc.tile([128, BT, K], f32, tag="ek")
                nc.vector.tensor_tensor(ek[:], numv[:], rden[:], Alu.mult)
                nc.scalar.activation(ek[:], ek[:], Act.Exp,
                                     bias=zero128[:], scale=1.0 / TAU)
                se = pc.tile([128, BT, 1], f32, tag="se")
                nc.vector.tensor_reduce(se[:], ek[:], axis=AX.X, op=Alu.add)
                rse = pc.tile([128, BT, 1], f32, tag="rse")
                nc.vector.reciprocal(rse[:], se[:])
                nc.vector.tensor_tensor(ek[:], ek[:],
                                        rse[:].broadcast_to((128, BT, K)),
                                        Alu.mult)
                uw16 = pc.tile([128, BT, K], f16, tag="uw16")
                nc.vector.tensor_tensor(
                    uw16[:], ek[:],
                    de16[:, b0:b0 + BT, :].broadcast_to((128, BT, K)), Alu.add)
                # weighted aggregation (fp16 2x tree)
                prod = pc.tile([128, BT, C, K], f16, tag="prod")
                nc.vector.tensor_tensor(
                    prod[:], fnei[:, :, 0:64, :],
                    uw16[:].unsqueeze(2).broadcast_to((128, BT, C, K)),
                    Alu.mult)
                s1t = pc.tile([128, BT, C, 8], f16, tag="s1t")
                nc.vector.tensor_tensor(s1t[:], prod[:, :, :, 0:8],
                                        prod[:, :, :, 8:16], Alu.add)
                s2t = pc.tile([128, BT, C, 4], f16, tag="s2t")
                nc.vector.tensor_tensor(s2t[:], s1t[:, :, :, 0:4],
                                        s1t[:, :, :, 4:8], Alu.add)
                s3t = pc.tile([128, BT, C, 2], f16, tag="s3t")
                nc.vector.tensor_tensor(s3t[:], s2t[:, :, :, 0:2],
                                        s2t[:, :, :, 2:4], Alu.add)
                nc.vector.tensor_tensor(agg[:, b0:b0 + BT, :],
                                        s3t[:, :, :, 0], s3t[:, :, :, 1],
                                        Alu.add)
                if debug:
                    uwf = pc.tile([128, BT, K], f32, tag="uwf")
                    nc.vector.tensor_copy(uwf[:], uw16[:])
                    nc.sync.dma_start(
                        AP(dbg_ext["d_uw"], b0 * K,
                           [[NB * K, 128], [K, BT], [1, K]]), uwf[:])
                    nc.sync.dma_start(
                        AP(dbg_ext["d_numv"], b0 * K,
                           [[NB * K, 128], [K, BT], [1, K]]), numv[:])
                # dist stats
                ndsl = dp[:, b0:b0 + BT, 0:1]
                nvsl = dp[:, b0:b0 + BT, 1:2]
                nc.vector.tensor_reduce(ndsl, sqd[:], axis=AX.X, op=Alu.add)
                nc.vector.tensor_scalar(ndsl, ndsl, 1.0 / K, None, Alu.mult)
                d2m = pc.tile([128, BT, 1], f32, tag="d2m")
                nc.vector.tensor_reduce(d2m[:], d2k[:], axis=AX.X, op=Alu.add)
                nc.vector.tensor_scalar(d2m[:], d2m[:], 1.0 / K, None, Alu.mult)
                nd2 = pc.tile([128, BT, 1], f32, tag="nd2")
                nc.vector.tensor_tensor(nd2[:], ndsl, ndsl, Alu.mult)
                nc.vector.tensor_tensor(nvsl, d2m[:], nd2[:], Alu.subtract)
                # dist stats -> headsT rows 3:5 (dpre/upre already consumed)
                sl = slice(512 * s, 512 * (s + 1))
                ptd = ps(2, 512)
                for q in range(BT):
                    nc.tensor.matmul(ptd[:, 128 * q:128 * (q + 1)],
                                     dp[:, b0 + q:b0 + q + 1, :], ident[:, :],
                                     is_transpose=True)
                nc.scalar.copy(headsT[3:5, sl], ptd[:])
                # reaction conv chunk
                px = ps(C, 512)
                nc.tensor.matmul(px[:], Wr1fT[:], fT[:, sl], start=True,
                                 stop=False)
                nc.tensor.matmul(px[:], Wr1hT[:], headsT[:, sl],
                                 start=False, stop=True)
                nc.scalar.activation(x_sb[:, sl], px[:], Act.Copy, bias=0.0,
                                     scale=1.0, accum_out=xs6[:, s:s + 1])
                sqscr = pc.tile([C, 512], f32, tag="sqscr")
                nc.scalar.activation(sqscr[:], x_sb[:, sl], Act.Square,
                                     bias=zero128[0:C, :], scale=1.0,
                                     accum_out=x2s6[:, s:s + 1])

            for b in range(NB):
                cand = small.tile([128, 64], f32, tag="cand", bufs=2)
                segloc = small.tile([128, 64], u16, tag="segloc", bufs=2)
                for h in range(2):
                    ngp = ngps.tile([128, 1536], f32, tag="ngp")
                    for j in range(3):
                        cj = 3 * h + j
                        nc.tensor.matmul(ngp[:, 512 * j:512 * (j + 1)],
                                         A36[:, 128 * b:128 * (b + 1)],
                                         B36[:, 512 * cj:512 * (cj + 1)],
                                         start=True, stop=True)
                    negd = ngsb.tile([128, 1536], f32, tag="negd")
                    nc.scalar.copy(negd[:], ngp[:])
                    for s in range(4):
                        s8 = 4 * h + s
                        nc.vector.max(cand[:, 8 * s8:8 * (s8 + 1)],
                                      negd[:, 384 * s:384 * (s + 1)])
                        nc.vector.max_index(segloc[:, 8 * s8:8 * (s8 + 1)],
                                            cand[:, 8 * s8:8 * (s8 + 1)],
                                            negd[:, 384 * s:384 * (s + 1)])
                jc16 = small.tile([128, 64], u16, tag="jc16", bufs=2)
                nc.vector.tensor_tensor(jc16[:], segloc[:], segb64u[:], Alu.add)
                v16 = v16_all[:, b, :]
                mrc = small.tile([128, 64], f32, tag="mrc", bufs=2)
                cp16 = small.tile([128, 16], u16, tag="cp16", bufs=2)
                nc.vector.max(v16[:, 0:8], cand[:])
                nc.vector.max_index(cp16[:, 0:8], v16[:, 0:8], cand[:])
                nc.vector.match_replace(mrc[:], v16[:, 0:8], cand[:], -1e30)
                nc.vector.max(v16[:, 8:16], mrc[:])
                nc.vector.max_index(cp16[:, 8:16], v16[:, 8:16], mrc[:])
                rankmap = small.tile([128, 64], i16, tag="rankmap", bufs=2)
                nc.gpsimd.local_scatter(rankmap[:], rank16[:],
                                        cp16[:].bitcast(i16),
                                        channels=128, num_elems=64, num_idxs=16)
                nc.vector.tensor_scalar(rankmap[:], rankmap[:], 1, None,
                                        Alu.subtract)
                nc.gpsimd.local_scatter(idx_all[:, K * b:K * (b + 1)].bitcast(i16),
                                        jc16[:].bitcast(i16), rankmap[:],
                                        channels=128, num_elems=16, num_idxs=64)

                if b % BT == BT - 1:
                    bg = b // BT
                    # stage idx to DRAM wrapped layout, read back, gather
                    # DRAM addr = bg*8192 + 512*(p%16) + 32*k + 8*bl + p//16
                    for bl in range(BT):
                        nc.sync.dma_start(
                            AP(idx_dram.tensor, bg * 512 * K + 8 * bl,
                               [[1, 8], [512, 16], [32, K]]),
                            idx_all[:, (b - BT + 1 + bl) * K:
                                    (b - BT + 2 + bl) * K].bitcast(i16))
                    for g in range(8):
                        nc.sync.dma_start(
                            idx_wrap[16 * g:16 * (g + 1), bg * K:(bg + 1) * K, :],
                            AP(idx_dram.tensor, bg * 512 * K,
                               [[512, 16], [32, K], [1, 32]]))
                    gth_h = []
                    for hh in range(2):
                        gthh = gpool.tile([128, 8 * BT * 128], f32, tag="gth")
                        gth_h.append(gthh)
                        nc.gpsimd.ap_gather(
                            gthh[:],
                            fxT[:],
                            idx_wrap[:, bg * K + 8 * hh:bg * K + 8 * (hh + 1),
                                     :].rearrange("p a q -> p (a q)"),
                            channels=128, num_elems=N, d=1,
                            num_idxs=8 * BT * 128)
                    gth_tiles[bg] = gth_h
                    if bg >= 1:
                        process_slice(bg - 1)
            process_slice(NSL - 1)

            if debug:
                nc.sync.dma_start(AP(dbg_ext["d_agg"], 0, [[NB * C, 128], [1, NB * C]]),
                                  agg[:])
                nc.sync.dma_start(AP(dbg_ext["d_v16"], 0, [[NB * K, 128], [1, NB * K]]),
                                  v16_all[:])
                idxf = latex.tile([128, NB * K], f32, tag="idxf")
                nc.vector.tensor_copy(idxf[:], idx_all[:])
                nc.sync.dma_start(AP(dbg_ext["d_idx"], 0, [[NB * K, 128], [1, NB * K]]),
                                  idxf[:])
                nc.sync.dma_start(AP(dbg_ext["d_heads"], 0, [[N, 5], [1, N]]),
                                  headsT[:])

            # ---------------- S3 BN + reaction tail + combine ------------
            loopstk.close()
            late = ctx.enter_context(tc.tile_pool(name="late", bufs=1))
            scr = late.tile([C, N], f32)
            bn_loc = small.tile([C, 2], f32)
            nc.vector.tensor_reduce(bn_loc[:, 0:1], xs6[:], axis=AX.X, op=Alu.add)
            nc.vector.tensor_reduce(bn_loc[:, 1:2], x2s6[:], axis=AX.X, op=Alu.add)
            bn_in = dram.tile([C, 2], f32)
            bn_out = dram.tile([C, 2], f32)
            bn_g = small.tile([C, 2], f32)
            if nocol:
                nc.vector.tensor_scalar(bn_g[:], bn_loc[:], float(NCORES), None,
                                        Alu.mult)
            else:
                nc.sync.dma_start(bn_in[:], bn_loc[:])
                nc.gpsimd.collective_compute(
                    "AllReduce", Alu.add, replica_groups=[list(range(NCORES))],
                    ins=[bn_in[:].opt()], outs=[bn_out[:].opt()])
                nc.sync.dma_start(bn_g[:], bn_out[:])
            Mtot = float(NCORES * N)
            mu = small.tile([C, 1], f32)
            nc.vector.tensor_scalar(mu[:], bn_g[:, 0:1], 1.0 / Mtot, None, Alu.mult)
            var = small.tile([C, 1], f32)
            nc.vector.tensor_scalar(var[:], bn_g[:, 1:2], 1.0 / Mtot, None, Alu.mult)
            mu2 = small.tile([C, 1], f32)
            nc.vector.tensor_tensor(mu2[:], mu[:], mu[:], Alu.mult)
            nc.vector.tensor_tensor(var[:], var[:], mu2[:], Alu.subtract)
            nc.vector.tensor_scalar(var[:], var[:], BN_EPS, None, Alu.add)
            nc.scalar.activation(var[:], var[:], Act.Sqrt,
                                 bias=zero128[0:C, :], scale=1.0)
            rstd = small.tile([C, 1], f32)
            nc.vector.reciprocal(rstd[:], var[:])
            s_vec = small.tile([C, 1], f32)
            nc.vector.tensor_tensor(s_vec[:], gamma_v[:], rstd[:], Alu.mult)
            b_vec = small.tile([C, 1], f32)
            nc.vector.tensor_tensor(b_vec[:], mu[:], s_vec[:], Alu.mult)
            nc.vector.tensor_tensor(b_vec[:], beta_v[:], b_vec[:], Alu.subtract)

            nc.scalar.activation(x_sb[:], x_sb[:], Act.Relu,
                                 bias=b_vec[:], scale=s_vec[:])
            for j in range(6):
                sl = slice(512 * j, 512 * (j + 1))
                pr = ps(C, 512)
                nc.tensor.matmul(pr[:], Wr2T[:], x_sb[:, sl], start=True, stop=True)
                nc.scalar.copy(scr[:, sl], pr[:])
            nc.vector.tensor_scalar(scr[:], scr[:], br2_v[:], None, Alu.add)

            # out = f*(1 - dt*(1+de)) + dt*(agg + TR + reac)
            de1 = small.tile([128, NB, 1], f32)
            nc.vector.tensor_scalar(de1[:], de[:], 1.0, None, Alu.add)
            alpha = small.tile([128, NB, 1], f32)
            nc.vector.tensor_scalar(alpha[:], de1[:], dtv[:, 0:1], None, Alu.mult)
            nc.vector.tensor_scalar(alpha[:], alpha[:], -1.0, 1.0, Alu.mult, Alu.add)
            fa = late.tile([128, NB, C], f32)
            nc.gpsimd.tensor_tensor(fa[:], f_sb[:],
                                    alpha[:].broadcast_to((128, NB, C)), Alu.mult)
            nc.gpsimd.tensor_tensor(agg[:], agg[:], TRp[:], Alu.add)
            out_sb = late.tile([128, NB, C], f32)
            for j in range(3):
                pt = ps(128, 512)
                for q in range(8):
                    b = 8 * j + q
                    nc.tensor.matmul(pt[:, C * q:C * (q + 1)],
                                     scr[:, 128 * b:128 * (b + 1)],
                                     ident[0:C, 0:C], is_transpose=True)
                nc.scalar.copy(out_sb[:, 8 * j:8 * (j + 1), :], pt[:])
            nc.vector.tensor_tensor(agg[:], agg[:], out_sb[:], Alu.add)
            nc.vector.scalar_tensor_tensor(out_sb[:], agg[:], dtv[:], fa[:],
                                           Alu.mult, Alu.add)
            nc.sync.dma_start(AP(out_ext, 0, [[C, 128], [128 * C, NB], [1, C]]),
                              out_sb[:])

    nc.compile()
    return nc


@functools.cache
def _get_nc(debug=False):
    return _build(debug=debug)


def _run(nc, inputs, trace=False):
    from concourse.bass_utils import run_bass_kernel_spmd
    f_seq = np.ascontiguousarray(np.asarray(inputs["f_seq"], dtype=np.float32))
    xyz = np.ascontiguousarray(np.asarray(inputs["xyz"], dtype=np.float32))
    in_maps = []
    for core in range(NCORES):
        b, l = divmod(core, L)
        m = {"f": f_seq[b, l], "xyz": xyz[b, l]}
        for k in WEIGHT_NAMES:
            m[k] = np.ascontiguousarray(
                np.asarray(inputs[k], dtype=np.float32).reshape(-1))
        in_maps.append(m)
    return run_bass_kernel_spmd(nc, in_maps, core_ids=list(range(NCORES)),
                                trace=trace)


def kernel(**inputs):
    nc = _get_nc()
    res = _run(nc, inputs)
    out = np.stack([np.asarray(res.results[i]["out"]) for i in range(NCORES)])
    return out.reshape(B, L, N, C).astype(np.float32)
